# revision 27
# baseline (speedup 1.0000x reference)
"""AttentionMPLayer on 8 Trainium2 NeuronCores (Bass/Tile).

Sharding: nodes in 8 contiguous blocks (12500/core); edges routed to the core
owning their src node.  Within a core edges are packed DENSELY (128 per
column, no per-row alignment), sorted by dst-core so each dma_gather call
reads one 12544-row table slab with int16 indices.

Per edge the device gathers a 256B table row [k48|lm|pad|h48|pad] (fp16
content, gathered as f32x64) and a 256B q row [q48|1|pad], computes
score = q~.(k~ + 0.1 ef), w = exp(score), and dma_scatter_adds [w*h | w]
(49 f32) into a per-src-node accumulator.  A tail phase normalizes
(agg = num/den), applies the output head (Wu1/Wu2 with Wm folded), leaky
relu, and a batched LayerNorm (rsqrt via exp(-0.5 ln)).

Kernel A computes k~ = LN(h@Wk.T) and q~ = LN(h@Wq.T) with the mean
subtraction folded into host-transformed weights W.T @ (I - J/48), variance
via fused multiply-reduce, and rstd via exp(-0.5 ln(var+eps)).
"""
import numpy as np

N, E, H, NC = 100000, 1600000, 48, 8
BLK = N // NC            # 12500 nodes per core
PG = 128
NGT = 98                 # tail groups (12544 = 128*98)
RPAD = PG * NGT          # 12544
SLAB = RPAD              # k-table rows per core slab
POISON = BLK             # first poison row within a slab
DUMP = RPAD - 1          # accumulator dump row for pad edges
EW = 64                  # f32 words per table row (= 128 fp16)
WW = 49                  # scatter payload f32 words [w*h(48) | w]
SGC = 48                 # columns (x128 edges) per supergroup
EPS_LN = 1e-5
EPS_DEN = 1e-30
LMNEG = -30000.0

_build_cache = {}


# ---------------------------------------------------------------- host routing
GCH = 8   # max columns (x128 descriptors) per gather/scatter call


def _chunk(lo, hi, bounds):
    """Split [lo,hi) at `bounds` and into <=GCH-col chunks."""
    cuts = sorted({lo, hi} | {b for b in bounds if lo < b < hi})
    out = []
    for a, b in zip(cuts[:-1], cuts[1:]):
        x = a
        while x < b:
            out.append((x, min(x + GCH, b)))
            x = min(x + GCH, b)
    return out


def _plan(src, dst):
    """Dense layout in (dst-core, occurrence-layer) cells, shared schedule.

    Within a cell every edge has a distinct src (occurrence index within
    (src, dst-core) is constant), so scatter-add calls confined to one cell
    have unique indices.  Cells are padded to 128-edge column boundaries.
    """
    percore = []
    cellcnt = {}   # (c, dc, k) -> count
    maxk = np.zeros(NC, np.int64)
    for c in range(NC):
        m = np.nonzero((src >= c * BLK) & (src < (c + 1) * BLK))[0]
        s_loc = src[m] - c * BLK
        dc = dst[m] // BLK
        order = np.lexsort((s_loc, dc))
        m = m[order]
        s_loc = s_loc[order]
        dc = dc[order]
        # occurrence index within (dc, src) runs (sorted, so runs contiguous)
        key = dc * BLK + s_loc
        first = np.searchsorted(key, key, side="left")
        k = np.arange(len(m)) - first
        order2 = np.lexsort((s_loc, k, dc))
        m = m[order2]
        k = k[order2]
        dc = dc[order2]
        percore.append((m, dc, k))
        for dcv in range(NC):
            sel = dc == dcv
            if sel.any():
                kk = k[sel]
                maxk[dcv] = max(maxk[dcv], kk.max() + 1)
                bc = np.bincount(kk)
                for kv, n in enumerate(bc):
                    if n:
                        cellcnt[(c, dcv, kv)] = int(n)
    # shared cell column counts
    cells = []          # ordered (dc, k, cols)
    for dcv in range(NC):
        for kv in range(int(maxk[dcv])):
            n = max(cellcnt.get((c, dcv, kv), 0) for c in range(NC))
            if n:
                cells.append((dcv, kv, (n + PG - 1) // PG))
    CO = {}
    off = 0
    seg_lo = {}
    seg_hi = {}
    for (dcv, kv, cols) in cells:
        CO[(dcv, kv)] = off
        seg_lo.setdefault(dcv, off)
        seg_hi[dcv] = off + cols
        off += cols
    TC = off
    cell_bounds = sorted(CO.values()) + [TC]

    sgs = []
    c0 = 0
    while c0 < TC:
        sgs.append((c0, min(SGC, TC - c0)))
        c0 += SGC
    kpieces, spieces, qpieces = [], [], []
    for (c0, ncs) in sgs:
        kp = []
        for dcv in range(NC):
            if dcv not in seg_lo:
                continue
            lo, hi = max(c0, seg_lo[dcv]), min(c0 + ncs, seg_hi[dcv])
            if lo < hi:
                kp.extend((dcv, a - c0, b - c0) for (a, b) in
                          _chunk(lo, hi, cell_bounds))
        kpieces.append(kp)
        spieces.append([(a - c0, b - c0) for (a, b) in
                        _chunk(c0, c0 + ncs, cell_bounds)])
        qpieces.append([(a - c0, b - c0) for (a, b) in
                        _chunk(c0, c0 + ncs, [])])
    return percore, cells, CO, TC, sgs, kpieces, spieces, qpieces


def _prep(inputs):
    h = np.asarray(inputs["h"], np.float32)
    ei = np.asarray(inputs["edge_index"])
    ea = np.asarray(inputs["edge_attr"], np.float32)
    nm = np.asarray(inputs["node_mult"], np.float32)
    src = ei[0].astype(np.int64)
    dst = ei[1].astype(np.int64)
    percore, cells, CO, TC, sgs, kpieces, spieces, qpieces = _plan(src, dst)

    def wrap16(flat, vals, pos):
        # idx for position i lives at [i % 16 + 16*q7core, i // 16]
        r, cc = pos % 16, pos // 16
        for q7 in range(8):
            flat[16 * q7 + r, cc] = vals

    in_b = []
    for c in range(NC):
        m, dc, k = percore[c]
        kix = np.full((PG, TC * 8), POISON, np.int16)
        six = np.full((PG, TC * 8), DUMP, np.int16)
        qix = np.zeros((PG, TC * 8), np.int16)
        ef2 = np.zeros((PG, TC, H), np.float16)
        pos = np.empty(len(m), np.int64)
        for (dcv, kv, cols) in cells:
            sel = np.nonzero((dc == dcv) & (k == kv))[0]
            pos[sel] = CO[(dcv, kv)] * PG + np.arange(len(sel))
        wrap16(kix, (dst[m] % BLK).astype(np.int16), pos)
        wrap16(qix, (src[m] - c * BLK).astype(np.int16), pos)
        wrap16(six, (src[m] - c * BLK).astype(np.int16), pos)
        ef2[pos % PG, pos // PG, 0:H] = (0.1 * ea[m]).astype(np.float16)
        # one combined idx stream per supergroup: [kix | qix | six] blocks
        idx3 = np.empty((PG, TC * 24), np.int16)
        for (c0, ncs) in sgs:
            blk = idx3[:, c0 * 24:(c0 + ncs) * 24]
            blk[:, 0:ncs * 8] = kix[:, c0 * 8:(c0 + ncs) * 8]
            blk[:, ncs * 8:ncs * 16] = qix[:, c0 * 8:(c0 + ncs) * 8]
            blk[:, ncs * 16:ncs * 24] = six[:, c0 * 8:(c0 + ncs) * 8]
        in_b.append(dict(idx3=idx3, ef2=ef2.reshape(PG, TC * H)))

    # tail inputs: node order on tile = row p*NGT + j
    hp16 = np.zeros((NC, RPAD, H), np.float16)
    for c in range(NC):
        hp16[c, :BLK] = h[c * BLK:(c + 1) * BLK].astype(np.float16)
    hp2 = np.ascontiguousarray(hp16.reshape(NC, PG, NGT * H))
    hpT = np.zeros((NC, H, RPAD), np.float16)
    for c in range(NC):
        # hpT[:, j*128 + p] = h[p*NGT + j]
        v = hp16[c].reshape(PG, NGT, H)          # [p, j, e]
        hpT[c] = np.ascontiguousarray(v.transpose(2, 1, 0).reshape(H, RPAD))

    # kernel A inputs
    hT = np.zeros((NC, H, RPAD), np.float16)
    nmt = np.ones((NC, PG, NGT), np.float32)
    for c in range(NC):
        blk = h[c * BLK:(c + 1) * BLK].astype(np.float16)  # [BLK, H]
        hT[c, :, :BLK] = blk.T
        tmp = np.ones(RPAD, np.float32)
        tmp[:BLK] = nm[c * BLK:(c + 1) * BLK]
        nmt[c] = tmp.reshape(NGT, PG).T  # A-tile [p, g] = node g*128+p
    return dict(h=h, TC=TC, sgs=sgs, kpieces=kpieces, spieces=spieces,
                qpieces=qpieces, in_b=in_b,
                hp2=hp2, hpT=hpT, hT=hT, nmt=nmt)


# ------------------------------------------------------------------- kernel A
def _build_a():
    if "A" in _build_cache:
        return _build_cache["A"]
    import concourse.bacc as bacc
    import concourse.tile as tile
    import concourse.mybir as mybir

    nc = bacc.Bacc("TRN2", target_bir_lowering=False, debug=False,
                   num_devices=NC)
    f32 = mybir.dt.float32
    f16 = mybir.dt.float16
    t_hT = nc.dram_tensor("hT", [H, RPAD], f16, kind="ExternalInput").ap()
    t_w2 = nc.dram_tensor("w2", [H, 2 * H], f16, kind="ExternalInput").ap()
    t_nm = nc.dram_tensor("nm", [PG, NGT], f32, kind="ExternalInput").ap()
    # tile layout [p, g*EW+e] = node g*128+p; host transposes
    t_kl = nc.dram_tensor("kl", [PG, NGT * EW], f16, kind="ExternalOutput").ap()
    t_qq = nc.dram_tensor("qq", [PG, NGT * EW], f16, kind="ExternalOutput").ap()

    mult = mybir.AluOpType.mult
    add = mybir.AluOpType.add
    AXX = mybir.AxisListType.X
    EXP = mybir.ActivationFunctionType.Exp
    LN_F = mybir.ActivationFunctionType.Ln
    COPY = mybir.ActivationFunctionType.Copy

    with tile.TileContext(nc) as tc, nc.allow_low_precision(reason="fp16 ln"):
        with tc.tile_pool(name="const", bufs=1) as cpool, \
             tc.tile_pool(name="work", bufs=2) as wpool, \
             tc.tile_pool(name="ps", bufs=4, space="PSUM") as ppool:
            w2_s = cpool.tile([H, 2 * H], f16)
            nc.sync.dma_start(out=w2_s[:], in_=t_w2)
            hT_s = cpool.tile([H, RPAD], f16)
            nc.sync.dma_start(out=hT_s[:], in_=t_hT)
            nm_s = cpool.tile([PG, NGT], f32)
            nc.sync.dma_start(out=nm_s[:], in_=t_nm)
            xc_s = cpool.tile([PG, NGT * 2 * H], f16)
            varT = cpool.tile([PG, 2 * NGT], f32)
            kl_s = cpool.tile([PG, NGT * EW], f16)
            qq_s = cpool.tile([PG, NGT * EW], f16)
            # pad columns [49:64) are stored to DRAM; zero them once
            nc.vector.memset(
                kl_s[:].rearrange("p (g e) -> p g e", g=NGT)[:, :, H + 1:EW], 0.0)
            nc.vector.memset(
                qq_s[:].rearrange("p (g e) -> p g e", g=NGT)[:, :, H + 1:EW], 0.0)

            for g in range(NGT):
                ps = ppool.tile([PG, 2 * H], f32, tag="ps")
                nc.tensor.matmul(out=ps[:], lhsT=hT_s[:, g * PG:(g + 1) * PG],
                                 rhs=w2_s[:], start=True, stop=True)
                xc = xc_s[:, g * 2 * H:(g + 1) * 2 * H]
                nc.scalar.activation(out=xc, in_=ps[:], func=COPY)
                sq = wpool.tile([PG, 2 * H], f16, tag="sq")
                nc.vector.tensor_tensor(out=sq[:], in0=xc, in1=xc, op=mult)
                nc.vector.tensor_reduce(
                    out=varT[:, 2 * g:2 * g + 2].unsqueeze(2),
                    in_=sq[:].rearrange("p (s e) -> p s e", s=2),
                    axis=AXX, op=add)
            # rstd = exp(-0.5 * ln(sumsq/H + eps))
            eps_s = cpool.tile([PG, 1], f32)
            nc.vector.memset(eps_s[:], EPS_LN)
            lv = cpool.tile([PG, 2 * NGT], f32)
            nc.scalar.activation(out=lv[:], in_=varT[:], func=LN_F,
                                 bias=eps_s[:], scale=1.0 / H)
            rstd = cpool.tile([PG, 2 * NGT], f32)
            nc.scalar.activation(out=rstd[:], in_=lv[:], func=EXP, scale=-0.5)
            # lm = ln(max(nm, 1))
            lmx = cpool.tile([PG, NGT], f32)
            nc.vector.tensor_scalar_max(lmx[:], nm_s[:], 1.0)
            lm = cpool.tile([PG, NGT], f32)
            nc.scalar.activation(out=lm[:], in_=lmx[:], func=LN_F)
            kl3 = kl_s[:].rearrange("p (g e) -> p g e", g=NGT)
            qq3 = qq_s[:].rearrange("p (g e) -> p g e", g=NGT)
            nc.vector.tensor_copy(kl3[:, :, H:H + 1], lm[:].unsqueeze(2))
            nc.vector.memset(qq3[:, :, H:H + 1], 1.0)
            for g in range(NGT):
                xc = xc_s[:, g * 2 * H:(g + 1) * 2 * H]
                nc.vector.tensor_scalar_mul(
                    kl_s[:, g * EW:g * EW + H], xc[:, 0:H],
                    rstd[:, 2 * g:2 * g + 1])
                nc.vector.tensor_scalar_mul(
                    qq_s[:, g * EW:g * EW + H], xc[:, H:2 * H],
                    rstd[:, 2 * g + 1:2 * g + 2])
            nc.sync.dma_start(out=t_kl, in_=kl_s[:])
            nc.sync.dma_start(out=t_qq, in_=qq_s[:])
    nc.compile()
    _build_cache["A"] = nc
    return nc


# ------------------------------------------------------------------- kernel B
def _build_b(TC, sgs, kpieces, spieces, qpieces):
    key = ("B", TC, tuple(sgs), str(kpieces), str(spieces), str(qpieces))
    if key in _build_cache:
        return _build_cache[key]
    import concourse.bacc as bacc
    import concourse.tile as tile
    import concourse.mybir as mybir
    from concourse.masks import make_identity
    from concourse import library_config

    nc = bacc.Bacc("TRN2", target_bir_lowering=False, debug=False,
                   num_devices=NC)
    f32 = mybir.dt.float32
    f16 = mybir.dt.float16
    i16 = mybir.dt.int16
    t_ktab = nc.dram_tensor("ktab", [NC * SLAB, EW], f32,
                            kind="ExternalInput").ap()
    t_qtab = nc.dram_tensor("qtab", [SLAB, EW], f32, kind="ExternalInput").ap()
    t_ef2 = nc.dram_tensor("ef2", [PG, TC * H], f16,
                           kind="ExternalInput").ap()
    t_idx3 = nc.dram_tensor("idx3", [PG, TC * 24], i16,
                            kind="ExternalInput").ap()
    t_wu1 = nc.dram_tensor("wu1", [H, H], f16, kind="ExternalInput").ap()
    t_wu2 = nc.dram_tensor("wu2", [H, H], f16, kind="ExternalInput").ap()
    t_hpT = nc.dram_tensor("hpT", [H, RPAD], f16, kind="ExternalInput").ap()
    t_hp2 = nc.dram_tensor("hp2", [PG, NGT * H], f16,
                           kind="ExternalInput").ap()
    t_out = nc.dram_tensor("out", [RPAD, H], f16, kind="ExternalOutput").ap()
    t_acc = nc.dram_tensor("acc", [RPAD, EW], f32, kind="Internal").ap()

    mult = mybir.AluOpType.mult
    add = mybir.AluOpType.add
    sub = mybir.AluOpType.subtract
    amax = mybir.AluOpType.max
    AXX = mybir.AxisListType.X
    EXP = mybir.ActivationFunctionType.Exp
    LN_F = mybir.ActivationFunctionType.Ln
    COPY = mybir.ActivationFunctionType.Copy

    with tile.TileContext(nc) as tc, nc.allow_low_precision(reason="fp16"):
        with tc.tile_pool(name="const", bufs=1) as cpool, \
             tc.tile_pool(name="idx", bufs=2) as ipool, \
             tc.tile_pool(name="gat", bufs=2) as gpool, \
             tc.tile_pool(name="wrk", bufs=2) as wpool, \
             tc.tile_pool(name="ps", bufs=4, space="PSUM") as ppool:
            nc.gpsimd.load_library(library_config.mlp)
            wu1_s = cpool.tile([H, H], f16)
            nc.sync.dma_start(out=wu1_s[:], in_=t_wu1)
            wu2_s = cpool.tile([H, H], f16)
            nc.sync.dma_start(out=wu2_s[:], in_=t_wu2)
            ident = cpool.tile([PG, PG], f16)
            make_identity(nc, ident)
            z_s = cpool.tile([PG, NGT * EW], f32)
            nc.vector.memset(z_s[:], 0.0)
            nc.scalar.dma_start(
                out=t_acc.rearrange("(p x) e -> p x e", p=PG),
                in_=z_s[:].rearrange("p (x e) -> p x e", e=EW))

            for si, (c0, ncs) in enumerate(sgs):
                idx_t = ipool.tile([PG, ncs * 24], i16, tag="idx")
                nc.scalar.dma_start(out=idx_t[:],
                                    in_=t_idx3[:, c0 * 24:(c0 + ncs) * 24])
                kix_t = idx_t[:, 0:ncs * 8]
                qix_t = idx_t[:, ncs * 8:ncs * 16]
                six_t = idx_t[:, ncs * 16:ncs * 24]
                ef_t = wpool.tile([PG, ncs * H], f16, tag="ef")
                nc.sync.dma_start(out=ef_t[:],
                                  in_=t_ef2[:, c0 * H:(c0 + ncs) * H])
                g_k = gpool.tile([PG, ncs * EW], f32, tag="gk")
                for (cp, r0, r1) in kpieces[si]:
                    nc.gpsimd.dma_gather(
                        out_ap=g_k[:, r0 * EW:r1 * EW].rearrange(
                            "p (x e) -> p x e", e=EW),
                        in_ap=t_ktab[cp * SLAB:(cp + 1) * SLAB, :],
                        idxs_ap=kix_t[:, r0 * 8:r1 * 8],
                        num_idxs=(r1 - r0) * PG,
                        num_idxs_reg=(r1 - r0) * PG,
                        elem_size=EW)
                g_q = gpool.tile([PG, ncs * EW], f32, tag="gq")
                for (r0, r1) in qpieces[si]:
                    nc.gpsimd.dma_gather(
                        out_ap=g_q[:, r0 * EW:r1 * EW].rearrange(
                            "p (x e) -> p x e", e=EW),
                        in_ap=t_qtab,
                        idxs_ap=qix_t[:, r0 * 8:r1 * 8],
                        num_idxs=(r1 - r0) * PG,
                        num_idxs_reg=(r1 - r0) * PG,
                        elem_size=EW)
                gk6 = g_k[:].bitcast(f16).rearrange("p (x e) -> p x e", e=2 * EW)
                gq6 = g_q[:].bitcast(f16).rearrange("p (x e) -> p x e", e=2 * EW)
                ef3 = ef_t[:].rearrange("p (x e) -> p x e", e=H)
                kef = wpool.tile([PG, ncs * H], f16, tag="kef")
                kef3 = kef[:].rearrange("p (x e) -> p x e", e=H)
                nc.vector.tensor_tensor(out=kef3, in0=gk6[:, :, 0:H],
                                        in1=ef3, op=add)
                prod = wpool.tile([PG, ncs * H], f16, tag="prod")
                prod3 = prod[:].rearrange("p (x e) -> p x e", e=H)
                nc.vector.tensor_tensor(out=prod3, in0=kef3,
                                        in1=gq6[:, :, 0:H], op=mult)
                score = wpool.tile([PG, ncs], f32, tag="score")
                nc.vector.tensor_reduce(out=score[:].unsqueeze(2), in_=prod3,
                                        axis=AXX, op=add)
                nc.vector.tensor_tensor(out=score[:].unsqueeze(2),
                                        in0=score[:].unsqueeze(2),
                                        in1=gk6[:, :, H:H + 1], op=add)
                esc = wpool.tile([PG, ncs], f32, tag="esc")
                nc.scalar.activation(out=esc[:], in_=score[:], func=EXP)
                w_t = wpool.tile([PG, ncs * WW], f32, tag="w")
                w3 = w_t[:].rearrange("p (x e) -> p x e", e=WW)
                nc.vector.tensor_tensor(
                    out=w3[:, :, 0:H], in0=gk6[:, :, EW:EW + H],
                    in1=esc[:].unsqueeze(2).to_broadcast([PG, ncs, H]),
                    op=mult)
                nc.vector.tensor_copy(w3[:, :, H:WW], esc[:].unsqueeze(2))
                for (r0, r1) in spieces[si]:
                    nc.gpsimd.dma_scatter_add(
                        out_ap=t_acc[:, 0:WW],
                        in_ap=w3[:, r0:r1, :],
                        idxs_ap=six_t[:, r0 * 8:r1 * 8],
                        num_idxs=(r1 - r0) * PG,
                        num_idxs_reg=(r1 - r0) * PG,
                        elem_size=WW,
                        elem_step=EW)

            # ------------------------------------------------------- tail
            acc_t = cpool.tile([PG, NGT * EW], f32)
            nc.sync.dma_start(
                out=acc_t[:].rearrange("p (x e) -> p x e", e=EW),
                in_=t_acc.rearrange("(p x) e -> p x e", p=PG))
            hpT_s = cpool.tile([H, RPAD], f16)
            nc.sync.dma_start(out=hpT_s[:], in_=t_hpT)
            hp2_s = cpool.tile([PG, NGT * H], f16)
            nc.sync.dma_start(out=hp2_s[:], in_=t_hp2)
            acc3 = acc_t[:].rearrange("p (x e) -> p x e", e=EW)
            den = cpool.tile([PG, NGT], f32)
            nc.vector.tensor_scalar_add(den[:].unsqueeze(2),
                                        acc3[:, :, H:H + 1], EPS_DEN)
            rin = cpool.tile([PG, NGT], f32)
            nc.vector.reciprocal(out=rin[:], in_=den[:])
            r16 = cpool.tile([PG, NGT * H], f16)
            sumT = cpool.tile([PG, NGT], f32)
            varT = cpool.tile([PG, NGT], f32)
            for j in range(NGT):
                agg16 = wpool.tile([PG, H], f16, tag="agg16")
                nc.vector.tensor_scalar_mul(
                    agg16[:], acc_t[:, j * EW:j * EW + H], rin[:, j:j + 1])
                aggT = ppool.tile([H, PG], f16, tag="aggT")
                nc.tensor.transpose(out=aggT[:], in_=agg16[:],
                                    identity=ident[:])
                aggTs = wpool.tile([H, PG], f16, tag="aggTs")
                nc.scalar.activation(out=aggTs[:], in_=aggT[:], func=COPY)
                zp = ppool.tile([PG, H], f32, tag="zp")
                nc.tensor.matmul(out=zp[:], lhsT=hpT_s[:, j * PG:(j + 1) * PG],
                                 rhs=wu1_s[:], start=True, stop=False)
                nc.tensor.matmul(out=zp[:], lhsT=aggTs[:], rhs=wu2_s[:],
                                 start=False, stop=True)
                zs = wpool.tile([PG, H], f16, tag="zs")
                nc.scalar.activation(out=zs[:], in_=zp[:], func=COPY,
                                     scale=0.01)
                z16 = wpool.tile([PG, H], f16, tag="z16")
                nc.vector.tensor_tensor(out=z16[:], in0=zp[:], in1=zs[:],
                                        op=amax)
                rj = r16[:, j * H:(j + 1) * H]
                nc.vector.tensor_tensor(out=rj, in0=z16[:],
                                        in1=hp2_s[:, j * H:(j + 1) * H],
                                        op=add)
                nc.vector.tensor_reduce(
                    out=sumT[:, j:j + 1].unsqueeze(2),
                    in_=rj.unsqueeze(1), axis=AXX, op=add)
                sq = wpool.tile([PG, H], f16, tag="sqt")
                nc.vector.tensor_tensor(out=sq[:], in0=rj, in1=rj, op=mult)
                nc.vector.tensor_reduce(
                    out=varT[:, j:j + 1].unsqueeze(2), in_=sq[:].unsqueeze(1),
                    axis=AXX, op=add)
            mean = cpool.tile([PG, NGT], f32)
            nc.vector.tensor_scalar_mul(mean[:], sumT[:], 1.0 / H)
            m2 = cpool.tile([PG, NGT], f32)
            nc.vector.tensor_tensor(out=m2[:], in0=mean[:], in1=mean[:],
                                    op=mult)
            var = cpool.tile([PG, NGT], f32)
            nc.vector.tensor_scalar_mul(var[:], varT[:], 1.0 / H)
            nc.vector.tensor_tensor(out=var[:], in0=var[:], in1=m2[:], op=sub)
            eps_s = cpool.tile([PG, 1], f32)
            nc.vector.memset(eps_s[:], EPS_LN)
            lv = cpool.tile([PG, NGT], f32)
            nc.scalar.activation(out=lv[:], in_=var[:], func=LN_F,
                                 bias=eps_s[:], scale=1.0)
            rstd = cpool.tile([PG, NGT], f32)
            nc.scalar.activation(out=rstd[:], in_=lv[:], func=EXP, scale=-0.5)
            nmr = cpool.tile([PG, NGT], f32)
            nc.vector.tensor_tensor(out=nmr[:], in0=mean[:], in1=rstd[:],
                                    op=mult)
            nc.vector.tensor_scalar_mul(nmr[:], nmr[:], -1.0)
            on = cpool.tile([PG, NGT * H], f16)
            for j in range(NGT):
                nc.vector.tensor_scalar(
                    out=on[:, j * H:(j + 1) * H], in0=r16[:, j * H:(j + 1) * H],
                    scalar1=rstd[:, j:j + 1], scalar2=nmr[:, j:j + 1],
                    op0=mult, op1=add)
            nc.sync.dma_start(
                out=t_out.rearrange("(p x) e -> p x e", p=PG),
                in_=on[:].rearrange("p (x e) -> p x e", e=H))
    nc.compile()
    _build_cache[key] = nc
    return nc


# -------------------------------------------------------------------- driver
def _make_maps(inputs, prep):
    h = prep["h"]
    wq = np.asarray(inputs["Wq"], np.float64)
    wk = np.asarray(inputs["Wk"], np.float64)
    wm = np.asarray(inputs["Wm"], np.float64)
    wu = np.asarray(inputs["Wu"], np.float64)
    gq = np.asarray(inputs["gq"], np.float32)
    bq = np.asarray(inputs["bq"], np.float32)
    gk = np.asarray(inputs["gk"], np.float32)
    bk = np.asarray(inputs["bk"], np.float32)
    go = np.asarray(inputs["go"], np.float32)
    bo = np.asarray(inputs["bo"], np.float32)
    triv = (np.all(gq == 1) and np.all(gk == 1) and np.all(go == 1)
            and np.all(bq == 0) and np.all(bk == 0) and np.all(bo == 0))
    assert triv, "non-trivial layernorm affine not implemented"

    cen = np.eye(H) - np.full((H, H), 1.0 / H)
    w2 = np.concatenate([wk.T @ cen, wq.T @ cen], axis=1).astype(np.float16)
    wu1 = np.ascontiguousarray(wu[:, :H].T).astype(np.float16)
    wu2 = np.ascontiguousarray((wu[:, H:] @ wm).T).astype(np.float16)

    maps_a = []
    for c in range(NC):
        maps_a.append(dict(hT=prep["hT"][c], w2=w2, nm=prep["nmt"][c]))
    return maps_a, wu1, wu2


def _make_tables(prep, res_a):
    """k-table [NC*SLAB, EW] f32-view (fp16 content) + per-core q tables."""
    h = prep["h"]

    def untile(t):  # [PG, NGT*EW] tile -> [RPAD, EW] rows (node g*128+p)
        return t.reshape(PG, NGT, EW).transpose(1, 0, 2).reshape(RPAD, EW)

    ktab16 = np.zeros((NC, SLAB, 2 * EW), np.float16)
    for c in range(NC):
        kl = untile(res_a[c]["kl"])              # [RPAD, EW] f16
        ktab16[c, :, 0:EW] = kl
        ktab16[c, :BLK, EW:EW + H] = h[c * BLK:(c + 1) * BLK].astype(np.float16)
        ktab16[c, BLK:, H] = LMNEG               # poison rows
    ktab = ktab16.reshape(NC * SLAB, 2 * EW).view(np.float32)
    qtabs = []
    for c in range(NC):
        qq = untile(res_a[c]["qq"])              # [RPAD, EW] f16
        q16 = np.zeros((SLAB, 2 * EW), np.float16)
        q16[:, 0:EW] = qq
        qtabs.append(q16.view(np.float32))
    return ktab, qtabs


def kernel(**inputs):
    from concourse.bass_utils import run_bass_kernel_spmd

    prep = _prep(inputs)
    maps_a, wu1, wu2 = _make_maps(inputs, prep)

    nc_a = _build_a()
    res_a = run_bass_kernel_spmd(nc_a, maps_a, core_ids=list(range(NC))).results

    ktab, qtabs = _make_tables(prep, res_a)

    nc_b = _build_b(prep["TC"], prep["sgs"], prep["kpieces"],
                    prep["spieces"], prep["qpieces"])
    maps_b = []
    for c in range(NC):
        m = dict(prep["in_b"][c])
        m["ktab"] = ktab
        m["qtab"] = qtabs[c]
        m["wu1"] = wu1
        m["wu2"] = wu2
        m["hpT"] = prep["hpT"][c]
        m["hp2"] = prep["hp2"][c]
        maps_b.append(m)
    res_b = run_bass_kernel_spmd(nc_b, maps_b, core_ids=list(range(NC))).results

    out = np.empty((N, H), np.float32)
    for c in range(NC):
        ob = res_b[c]["out"].astype(np.float32)  # [RPAD, H], row = node p*NGT+j
        out[c * BLK:(c + 1) * BLK] = ob[:BLK]
    return out


# revision 34
# speedup vs baseline: 1.1420x; 1.1420x over previous
"""AttentionMPLayer on 8 Trainium2 NeuronCores (Bass/Tile).

Sharding: nodes in 8 contiguous blocks (12500/core); edges routed to the core
owning their src node.  Within a core edges are packed DENSELY (128 per
column, no per-row alignment), sorted by dst-core so each dma_gather call
reads one 12544-row table slab with int16 indices.

Per edge the device gathers a 256B table row [k48|lm|pad|h48|pad] (fp16
content, gathered as f32x64) and a 256B q row [q48|1|pad], computes
score = q~.(k~ + 0.1 ef), w = exp(score), and dma_scatter_adds [w*h | w]
(49 f32) into a per-src-node accumulator.  A tail phase normalizes
(agg = num/den), applies the output head (Wu1/Wu2 with Wm folded), leaky
relu, and a batched LayerNorm (rsqrt via exp(-0.5 ln)).

Kernel A computes k~ = LN(h@Wk.T) and q~ = LN(h@Wq.T) with the mean
subtraction folded into host-transformed weights W.T @ (I - J/48), variance
via fused multiply-reduce, and rstd via exp(-0.5 ln(var+eps)).
"""
import numpy as np

N, E, H, NC = 100000, 1600000, 48, 8
BLK = N // NC            # 12500 nodes per core
PG = 128
NGT = 98                 # tail groups (12544 = 128*98)
RPAD = PG * NGT          # 12544
SLAB = RPAD              # k-table rows per core slab
POISON = BLK             # first poison row within a slab
DUMP = RPAD - 1          # accumulator dump row for pad edges
EW = 64                  # f32 words per table row (= 128 fp16)
WW = 49                  # scatter payload f32 words [w*h(48) | w]
SGC = 32                 # columns (x128 edges) per supergroup
EPS_LN = 1e-5
EPS_DEN = 1e-30
LMNEG = -30000.0

_build_cache = {}


# ---------------------------------------------------------------- host routing
GCH = 8   # max columns (x128 descriptors) per gather/scatter call


def _chunk(lo, hi, bounds):
    """Split [lo,hi) at `bounds` and into <=GCH-col chunks."""
    cuts = sorted({lo, hi} | {b for b in bounds if lo < b < hi})
    out = []
    for a, b in zip(cuts[:-1], cuts[1:]):
        x = a
        while x < b:
            out.append((x, min(x + GCH, b)))
            x = min(x + GCH, b)
    return out


def _plan(src, dst):
    """Dense layout in (dst-core, occurrence-layer) cells, shared schedule.

    Within a cell every edge has a distinct src (occurrence index within
    (src, dst-core) is constant), so scatter-add calls confined to one cell
    have unique indices.  Cells are padded to 128-edge column boundaries.
    """
    percore = []
    cellcnt = {}   # (c, dc, k) -> count
    maxk = np.zeros(NC, np.int64)
    for c in range(NC):
        m = np.nonzero((src >= c * BLK) & (src < (c + 1) * BLK))[0]
        s_loc = src[m] - c * BLK
        dc = dst[m] // BLK
        order = np.lexsort((s_loc, dc))
        m = m[order]
        s_loc = s_loc[order]
        dc = dc[order]
        # occurrence index within (dc, src) runs (sorted, so runs contiguous)
        key = dc * BLK + s_loc
        first = np.searchsorted(key, key, side="left")
        k = np.arange(len(m)) - first
        order2 = np.lexsort((s_loc, k, dc))
        m = m[order2]
        k = k[order2]
        dc = dc[order2]
        percore.append((m, dc, k))
        for dcv in range(NC):
            sel = dc == dcv
            if sel.any():
                kk = k[sel]
                maxk[dcv] = max(maxk[dcv], kk.max() + 1)
                bc = np.bincount(kk)
                for kv, n in enumerate(bc):
                    if n:
                        cellcnt[(c, dcv, kv)] = int(n)
    # shared cell column counts
    cells = []          # ordered (dc, k, cols)
    for dcv in range(NC):
        for kv in range(int(maxk[dcv])):
            n = max(cellcnt.get((c, dcv, kv), 0) for c in range(NC))
            if n:
                cells.append((dcv, kv, (n + PG - 1) // PG))
    CO = {}
    off = 0
    seg_lo = {}
    seg_hi = {}
    for (dcv, kv, cols) in cells:
        CO[(dcv, kv)] = off
        seg_lo.setdefault(dcv, off)
        seg_hi[dcv] = off + cols
        off += cols
    TC = off
    cell_bounds = sorted(CO.values()) + [TC]

    sgs = []
    c0 = 0
    while c0 < TC:
        sgs.append((c0, min(SGC, TC - c0)))
        c0 += SGC
    kpieces, spieces, qpieces = [], [], []
    for (c0, ncs) in sgs:
        kp = []
        for dcv in range(NC):
            if dcv not in seg_lo:
                continue
            lo, hi = max(c0, seg_lo[dcv]), min(c0 + ncs, seg_hi[dcv])
            if lo < hi:
                kp.extend((dcv, a - c0, b - c0) for (a, b) in
                          _chunk(lo, hi, cell_bounds))
        kpieces.append(kp)
        spieces.append([(a - c0, b - c0) for (a, b) in
                        _chunk(c0, c0 + ncs, cell_bounds)])
        qpieces.append([(a - c0, b - c0) for (a, b) in
                        _chunk(c0, c0 + ncs, [])])
    return percore, cells, CO, TC, sgs, kpieces, spieces, qpieces


def _prep(inputs):
    h = np.asarray(inputs["h"], np.float32)
    ei = np.asarray(inputs["edge_index"])
    ea = np.asarray(inputs["edge_attr"], np.float32)
    nm = np.asarray(inputs["node_mult"], np.float32)
    src = ei[0].astype(np.int64)
    dst = ei[1].astype(np.int64)
    percore, cells, CO, TC, sgs, kpieces, spieces, qpieces = _plan(src, dst)

    def wrap16(flat, vals, pos):
        # idx for position i lives at [i % 16 + 16*q7core, i // 16]
        r, cc = pos % 16, pos // 16
        for q7 in range(8):
            flat[16 * q7 + r, cc] = vals

    in_b = []
    for c in range(NC):
        m, dc, k = percore[c]
        kix = np.full((PG, TC * 8), POISON, np.int16)
        six = np.full((PG, TC * 8), DUMP, np.int16)
        qix = np.zeros((PG, TC * 8), np.int16)
        ef2 = np.zeros((PG, TC, H), np.float16)
        pos = np.empty(len(m), np.int64)
        for (dcv, kv, cols) in cells:
            sel = np.nonzero((dc == dcv) & (k == kv))[0]
            pos[sel] = CO[(dcv, kv)] * PG + np.arange(len(sel))
        wrap16(kix, (dst[m] % BLK).astype(np.int16), pos)
        wrap16(qix, (src[m] - c * BLK).astype(np.int16), pos)
        wrap16(six, (src[m] - c * BLK).astype(np.int16), pos)
        ef2[pos % PG, pos // PG, 0:H] = (0.1 * ea[m]).astype(np.float16)
        # one combined idx stream per supergroup: [kix | qix | six] blocks
        idx3 = np.empty((PG, TC * 24), np.int16)
        for (c0, ncs) in sgs:
            blk = idx3[:, c0 * 24:(c0 + ncs) * 24]
            blk[:, 0:ncs * 8] = kix[:, c0 * 8:(c0 + ncs) * 8]
            blk[:, ncs * 8:ncs * 16] = qix[:, c0 * 8:(c0 + ncs) * 8]
            blk[:, ncs * 16:ncs * 24] = six[:, c0 * 8:(c0 + ncs) * 8]
        in_b.append(dict(idx3=idx3, ef2=ef2.reshape(PG, TC * H)))

    # tail inputs: node order on tile = row p*NGT + j
    hp16 = np.zeros((NC, RPAD, H), np.float16)
    for c in range(NC):
        hp16[c, :BLK] = h[c * BLK:(c + 1) * BLK].astype(np.float16)
    hp2 = np.ascontiguousarray(hp16.reshape(NC, PG, NGT * H))
    hpT = np.zeros((NC, H, RPAD), np.float16)
    for c in range(NC):
        # hpT[:, j*128 + p] = h[p*NGT + j]
        v = hp16[c].reshape(PG, NGT, H)          # [p, j, e]
        hpT[c] = np.ascontiguousarray(v.transpose(2, 1, 0).reshape(H, RPAD))

    # kernel A inputs
    hT = np.zeros((NC, H, RPAD), np.float16)
    nmt = np.ones((NC, PG, NGT), np.float32)
    for c in range(NC):
        blk = h[c * BLK:(c + 1) * BLK].astype(np.float16)  # [BLK, H]
        hT[c, :, :BLK] = blk.T
        tmp = np.ones(RPAD, np.float32)
        tmp[:BLK] = nm[c * BLK:(c + 1) * BLK]
        nmt[c] = tmp.reshape(NGT, PG).T  # A-tile [p, g] = node g*128+p
    return dict(h=h, TC=TC, sgs=sgs, kpieces=kpieces, spieces=spieces,
                qpieces=qpieces, in_b=in_b,
                hp2=hp2, hpT=hpT, hT=hT, nmt=nmt)


# ------------------------------------------------------------------- kernel A
def _build_a():
    if "A" in _build_cache:
        return _build_cache["A"]
    import concourse.bacc as bacc
    import concourse.tile as tile
    import concourse.mybir as mybir

    nc = bacc.Bacc("TRN2", target_bir_lowering=False, debug=False,
                   num_devices=NC)
    f32 = mybir.dt.float32
    f16 = mybir.dt.float16
    t_hT = nc.dram_tensor("hT", [H, RPAD], f16, kind="ExternalInput").ap()
    t_w2 = nc.dram_tensor("w2", [H, 2 * H], f16, kind="ExternalInput").ap()
    t_nm = nc.dram_tensor("nm", [PG, NGT], f32, kind="ExternalInput").ap()
    # tile layout [p, g*EW+e] = node g*128+p; host transposes
    t_kl = nc.dram_tensor("kl", [PG, NGT * EW], f16, kind="ExternalOutput").ap()
    t_qq = nc.dram_tensor("qq", [PG, NGT * EW], f16, kind="ExternalOutput").ap()

    mult = mybir.AluOpType.mult
    add = mybir.AluOpType.add
    AXX = mybir.AxisListType.X
    EXP = mybir.ActivationFunctionType.Exp
    LN_F = mybir.ActivationFunctionType.Ln
    COPY = mybir.ActivationFunctionType.Copy

    with tile.TileContext(nc) as tc, nc.allow_low_precision(reason="fp16 ln"):
        with tc.tile_pool(name="const", bufs=1) as cpool, \
             tc.tile_pool(name="work", bufs=2) as wpool, \
             tc.tile_pool(name="ps", bufs=4, space="PSUM") as ppool:
            w2_s = cpool.tile([H, 2 * H], f16)
            nc.sync.dma_start(out=w2_s[:], in_=t_w2)
            hT_s = cpool.tile([H, RPAD], f16)
            nc.sync.dma_start(out=hT_s[:], in_=t_hT)
            nm_s = cpool.tile([PG, NGT], f32)
            nc.sync.dma_start(out=nm_s[:], in_=t_nm)
            xc_s = cpool.tile([PG, NGT * 2 * H], f16)
            varT = cpool.tile([PG, 2 * NGT], f32)
            kl_s = cpool.tile([PG, NGT * EW], f16)
            qq_s = cpool.tile([PG, NGT * EW], f16)
            # pad columns [49:64) are stored to DRAM; zero them once
            nc.vector.memset(
                kl_s[:].rearrange("p (g e) -> p g e", g=NGT)[:, :, H + 1:EW], 0.0)
            nc.vector.memset(
                qq_s[:].rearrange("p (g e) -> p g e", g=NGT)[:, :, H + 1:EW], 0.0)

            for g in range(NGT):
                ps = ppool.tile([PG, 2 * H], f32, tag="ps")
                nc.tensor.matmul(out=ps[:], lhsT=hT_s[:, g * PG:(g + 1) * PG],
                                 rhs=w2_s[:], start=True, stop=True)
                xc = xc_s[:, g * 2 * H:(g + 1) * 2 * H]
                nc.scalar.activation(out=xc, in_=ps[:], func=COPY)
                sq = wpool.tile([PG, 2 * H], f16, tag="sq")
                nc.vector.tensor_tensor(out=sq[:], in0=xc, in1=xc, op=mult)
                nc.vector.tensor_reduce(
                    out=varT[:, 2 * g:2 * g + 2].unsqueeze(2),
                    in_=sq[:].rearrange("p (s e) -> p s e", s=2),
                    axis=AXX, op=add)
            # rstd = exp(-0.5 * ln(sumsq/H + eps))
            eps_s = cpool.tile([PG, 1], f32)
            nc.vector.memset(eps_s[:], EPS_LN)
            lv = cpool.tile([PG, 2 * NGT], f32)
            nc.scalar.activation(out=lv[:], in_=varT[:], func=LN_F,
                                 bias=eps_s[:], scale=1.0 / H)
            rstd = cpool.tile([PG, 2 * NGT], f32)
            nc.scalar.activation(out=rstd[:], in_=lv[:], func=EXP, scale=-0.5)
            # lm = ln(max(nm, 1))
            lmx = cpool.tile([PG, NGT], f32)
            nc.vector.tensor_scalar_max(lmx[:], nm_s[:], 1.0)
            lm = cpool.tile([PG, NGT], f32)
            nc.scalar.activation(out=lm[:], in_=lmx[:], func=LN_F)
            kl3 = kl_s[:].rearrange("p (g e) -> p g e", g=NGT)
            qq3 = qq_s[:].rearrange("p (g e) -> p g e", g=NGT)
            nc.vector.tensor_copy(kl3[:, :, H:H + 1], lm[:].unsqueeze(2))
            nc.vector.memset(qq3[:, :, H:H + 1], 1.0)
            for g in range(NGT):
                xc = xc_s[:, g * 2 * H:(g + 1) * 2 * H]
                nc.vector.tensor_scalar_mul(
                    kl_s[:, g * EW:g * EW + H], xc[:, 0:H],
                    rstd[:, 2 * g:2 * g + 1])
                nc.vector.tensor_scalar_mul(
                    qq_s[:, g * EW:g * EW + H], xc[:, H:2 * H],
                    rstd[:, 2 * g + 1:2 * g + 2])
            nc.sync.dma_start(out=t_kl, in_=kl_s[:])
            nc.sync.dma_start(out=t_qq, in_=qq_s[:])
    nc.compile()
    _build_cache["A"] = nc
    return nc


# ------------------------------------------------------------------- kernel B
def _build_b(TC, sgs, kpieces, spieces, qpieces):
    key = ("B", TC, tuple(sgs), str(kpieces), str(spieces), str(qpieces))
    if key in _build_cache:
        return _build_cache[key]
    import concourse.bacc as bacc
    import concourse.tile as tile
    import concourse.mybir as mybir
    from concourse.masks import make_identity
    from concourse import library_config

    nc = bacc.Bacc("TRN2", target_bir_lowering=False, debug=False,
                   num_devices=NC)
    f32 = mybir.dt.float32
    f16 = mybir.dt.float16
    i16 = mybir.dt.int16
    t_ktab = nc.dram_tensor("ktab", [NC * SLAB, EW], f32,
                            kind="ExternalInput").ap()
    t_qtab = nc.dram_tensor("qtab", [SLAB, EW], f32, kind="ExternalInput").ap()
    t_ef2 = nc.dram_tensor("ef2", [PG, TC * H], f16,
                           kind="ExternalInput").ap()
    t_idx3 = nc.dram_tensor("idx3", [PG, TC * 24], i16,
                            kind="ExternalInput").ap()
    t_wu1 = nc.dram_tensor("wu1", [H, H], f16, kind="ExternalInput").ap()
    t_wu2 = nc.dram_tensor("wu2", [H, H], f16, kind="ExternalInput").ap()
    t_hpT = nc.dram_tensor("hpT", [H, RPAD], f16, kind="ExternalInput").ap()
    t_hp2 = nc.dram_tensor("hp2", [PG, NGT * H], f16,
                           kind="ExternalInput").ap()
    t_out = nc.dram_tensor("out", [RPAD, H], f16, kind="ExternalOutput").ap()
    t_acc = nc.dram_tensor("acc", [RPAD, EW], f32, kind="Internal").ap()

    mult = mybir.AluOpType.mult
    add = mybir.AluOpType.add
    sub = mybir.AluOpType.subtract
    amax = mybir.AluOpType.max
    AXX = mybir.AxisListType.X
    EXP = mybir.ActivationFunctionType.Exp
    LN_F = mybir.ActivationFunctionType.Ln
    COPY = mybir.ActivationFunctionType.Copy

    with tile.TileContext(nc) as tc, nc.allow_low_precision(reason="fp16"):
        with tc.tile_pool(name="const", bufs=1) as cpool, \
             tc.tile_pool(name="idx", bufs=3) as ipool, \
             tc.tile_pool(name="gat", bufs=3) as gpool, \
             tc.tile_pool(name="wrk", bufs=3) as wpool, \
             tc.tile_pool(name="tl", bufs=6) as tpool, \
             tc.tile_pool(name="ps", bufs=4, space="PSUM") as ppool:
            nc.gpsimd.load_library(library_config.mlp)
            wu1_s = cpool.tile([H, H], f16)
            nc.sync.dma_start(out=wu1_s[:], in_=t_wu1)
            wu2_s = cpool.tile([H, H], f16)
            nc.sync.dma_start(out=wu2_s[:], in_=t_wu2)
            ident = cpool.tile([PG, PG], f16)
            make_identity(nc, ident)
            NZ = NGT * EW // 7
            z_s = cpool.tile([PG, NZ], f32)
            nc.vector.memset(z_s[:], 0.0)
            accv = t_acc.rearrange("(p q x) e -> p q (x e)", p=PG, q=7)
            for qq in range(7):
                nc.scalar.dma_start(out=accv[:, qq, :], in_=z_s[:])

            for si, (c0, ncs) in enumerate(sgs):
                idx_t = ipool.tile([PG, ncs * 24], i16, tag="idx")
                nc.scalar.dma_start(out=idx_t[:],
                                    in_=t_idx3[:, c0 * 24:(c0 + ncs) * 24])
                kix_t = idx_t[:, 0:ncs * 8]
                qix_t = idx_t[:, ncs * 8:ncs * 16]
                six_t = idx_t[:, ncs * 16:ncs * 24]
                ef_t = wpool.tile([PG, ncs * H], f16, tag="ef")
                nc.sync.dma_start(out=ef_t[:],
                                  in_=t_ef2[:, c0 * H:(c0 + ncs) * H])
                g_k = gpool.tile([PG, ncs * EW], f32, tag="gk")
                for (cp, r0, r1) in kpieces[si]:
                    nc.gpsimd.dma_gather(
                        out_ap=g_k[:, r0 * EW:r1 * EW].rearrange(
                            "p (x e) -> p x e", e=EW),
                        in_ap=t_ktab[cp * SLAB:(cp + 1) * SLAB, :],
                        idxs_ap=kix_t[:, r0 * 8:r1 * 8],
                        num_idxs=(r1 - r0) * PG,
                        num_idxs_reg=(r1 - r0) * PG,
                        elem_size=EW)
                g_q = gpool.tile([PG, ncs * EW], f32, tag="gq")
                for (r0, r1) in qpieces[si]:
                    nc.gpsimd.dma_gather(
                        out_ap=g_q[:, r0 * EW:r1 * EW].rearrange(
                            "p (x e) -> p x e", e=EW),
                        in_ap=t_qtab,
                        idxs_ap=qix_t[:, r0 * 8:r1 * 8],
                        num_idxs=(r1 - r0) * PG,
                        num_idxs_reg=(r1 - r0) * PG,
                        elem_size=EW)
                gk6 = g_k[:].bitcast(f16).rearrange("p (x e) -> p x e", e=2 * EW)
                gq6 = g_q[:].bitcast(f16).rearrange("p (x e) -> p x e", e=2 * EW)
                ef3 = ef_t[:].rearrange("p (x e) -> p x e", e=H)
                kef = wpool.tile([PG, ncs * H], f16, tag="kef")
                kef3 = kef[:].rearrange("p (x e) -> p x e", e=H)
                nc.vector.tensor_tensor(out=kef3, in0=gk6[:, :, 0:H],
                                        in1=ef3, op=add)
                prod = wpool.tile([PG, ncs * H], f16, tag="prod")
                prod3 = prod[:].rearrange("p (x e) -> p x e", e=H)
                nc.vector.tensor_tensor(out=prod3, in0=kef3,
                                        in1=gq6[:, :, 0:H], op=mult)
                score = wpool.tile([PG, ncs], f32, tag="score")
                nc.vector.tensor_reduce(out=score[:].unsqueeze(2), in_=prod3,
                                        axis=AXX, op=add)
                nc.vector.tensor_tensor(out=score[:].unsqueeze(2),
                                        in0=score[:].unsqueeze(2),
                                        in1=gk6[:, :, H:H + 1], op=add)
                esc = wpool.tile([PG, ncs], f32, tag="esc")
                nc.scalar.activation(out=esc[:], in_=score[:], func=EXP)
                w_t = wpool.tile([PG, ncs * WW], f32, tag="w")
                w3 = w_t[:].rearrange("p (x e) -> p x e", e=WW)
                nc.vector.tensor_tensor(
                    out=w3[:, :, 0:H], in0=gk6[:, :, EW:EW + H],
                    in1=esc[:].unsqueeze(2).to_broadcast([PG, ncs, H]),
                    op=mult)
                nc.vector.tensor_copy(w3[:, :, H:WW], esc[:].unsqueeze(2))
                for (r0, r1) in spieces[si]:
                    nc.gpsimd.dma_scatter_add(
                        out_ap=t_acc[:, 0:WW],
                        in_ap=w3[:, r0:r1, :],
                        idxs_ap=six_t[:, r0 * 8:r1 * 8],
                        num_idxs=(r1 - r0) * PG,
                        num_idxs_reg=(r1 - r0) * PG,
                        elem_size=WW,
                        elem_step=EW)

            # ------------------------------------------------------- tail
            acc_t = cpool.tile([PG, NGT * EW], f32)
            nc.sync.dma_start(
                out=acc_t[:].rearrange("p (x e) -> p x e", e=EW),
                in_=t_acc.rearrange("(p x) e -> p x e", p=PG))
            hpT_s = cpool.tile([H, RPAD], f16)
            nc.sync.dma_start(out=hpT_s[:], in_=t_hpT)
            hp2_s = cpool.tile([PG, NGT * H], f16)
            nc.sync.dma_start(out=hp2_s[:], in_=t_hp2)
            acc3 = acc_t[:].rearrange("p (x e) -> p x e", e=EW)
            den = cpool.tile([PG, NGT], f32)
            nc.vector.tensor_scalar_add(den[:].unsqueeze(2),
                                        acc3[:, :, H:H + 1], EPS_DEN)
            rin = cpool.tile([PG, NGT], f32)
            nc.vector.reciprocal(out=rin[:], in_=den[:])
            r16 = cpool.tile([PG, NGT * H], f16)
            sumT = cpool.tile([PG, NGT], f32)
            varT = cpool.tile([PG, NGT], f32)
            for j in range(NGT):
                agg16 = tpool.tile([PG, H], f16, tag="agg16")
                nc.vector.tensor_scalar_mul(
                    agg16[:], acc_t[:, j * EW:j * EW + H], rin[:, j:j + 1])
                aggT = ppool.tile([H, PG], f16, tag="aggT")
                nc.tensor.transpose(out=aggT[:], in_=agg16[:],
                                    identity=ident[:])
                aggTs = tpool.tile([H, PG], f16, tag="aggTs")
                nc.scalar.activation(out=aggTs[:], in_=aggT[:], func=COPY)
                zp = ppool.tile([PG, H], f32, tag="zp")
                nc.tensor.matmul(out=zp[:], lhsT=hpT_s[:, j * PG:(j + 1) * PG],
                                 rhs=wu1_s[:], start=True, stop=False)
                nc.tensor.matmul(out=zp[:], lhsT=aggTs[:], rhs=wu2_s[:],
                                 start=False, stop=True)
                zs = tpool.tile([PG, H], f16, tag="zs")
                nc.scalar.activation(out=zs[:], in_=zp[:], func=COPY,
                                     scale=0.01)
                z16 = tpool.tile([PG, H], f16, tag="z16")
                nc.vector.tensor_tensor(out=z16[:], in0=zp[:], in1=zs[:],
                                        op=amax)
                rj = r16[:, j * H:(j + 1) * H]
                nc.vector.tensor_tensor(out=rj, in0=z16[:],
                                        in1=hp2_s[:, j * H:(j + 1) * H],
                                        op=add)
                dmy = tpool.tile([PG, H], f16, tag="dmy")
                nc.vector.tensor_scalar(
                    out=dmy[:], in0=rj, scalar1=1.0, scalar2=0.0,
                    op0=mult, op1=add, accum_out=sumT[:, j:j + 1])
                sq = tpool.tile([PG, H], f16, tag="sqt")
                nc.vector.tensor_tensor(out=sq[:], in0=rj, in1=rj, op=mult)
                nc.vector.tensor_scalar(
                    out=dmy[:], in0=sq[:], scalar1=1.0, scalar2=0.0,
                    op0=mult, op1=add, accum_out=varT[:, j:j + 1])
            mean = cpool.tile([PG, NGT], f32)
            nc.vector.tensor_scalar_mul(mean[:], sumT[:], 1.0 / H)
            m2 = cpool.tile([PG, NGT], f32)
            nc.vector.tensor_tensor(out=m2[:], in0=mean[:], in1=mean[:],
                                    op=mult)
            var = cpool.tile([PG, NGT], f32)
            nc.vector.tensor_scalar_mul(var[:], varT[:], 1.0 / H)
            nc.vector.tensor_tensor(out=var[:], in0=var[:], in1=m2[:], op=sub)
            eps_s = cpool.tile([PG, 1], f32)
            nc.vector.memset(eps_s[:], EPS_LN)
            lv = cpool.tile([PG, NGT], f32)
            nc.scalar.activation(out=lv[:], in_=var[:], func=LN_F,
                                 bias=eps_s[:], scale=1.0)
            rstd = cpool.tile([PG, NGT], f32)
            nc.scalar.activation(out=rstd[:], in_=lv[:], func=EXP, scale=-0.5)
            nmr = cpool.tile([PG, NGT], f32)
            nc.vector.tensor_tensor(out=nmr[:], in0=mean[:], in1=rstd[:],
                                    op=mult)
            nc.vector.tensor_scalar_mul(nmr[:], nmr[:], -1.0)
            on = cpool.tile([PG, NGT * H], f16)
            for j in range(NGT):
                nc.vector.tensor_scalar(
                    out=on[:, j * H:(j + 1) * H], in0=r16[:, j * H:(j + 1) * H],
                    scalar1=rstd[:, j:j + 1], scalar2=nmr[:, j:j + 1],
                    op0=mult, op1=add)
            nc.sync.dma_start(
                out=t_out.rearrange("(p x) e -> p x e", p=PG),
                in_=on[:].rearrange("p (x e) -> p x e", e=H))
    nc.compile()
    _build_cache[key] = nc
    return nc


# -------------------------------------------------------------------- driver
def _make_maps(inputs, prep):
    h = prep["h"]
    wq = np.asarray(inputs["Wq"], np.float64)
    wk = np.asarray(inputs["Wk"], np.float64)
    wm = np.asarray(inputs["Wm"], np.float64)
    wu = np.asarray(inputs["Wu"], np.float64)
    gq = np.asarray(inputs["gq"], np.float32)
    bq = np.asarray(inputs["bq"], np.float32)
    gk = np.asarray(inputs["gk"], np.float32)
    bk = np.asarray(inputs["bk"], np.float32)
    go = np.asarray(inputs["go"], np.float32)
    bo = np.asarray(inputs["bo"], np.float32)
    triv = (np.all(gq == 1) and np.all(gk == 1) and np.all(go == 1)
            and np.all(bq == 0) and np.all(bk == 0) and np.all(bo == 0))
    assert triv, "non-trivial layernorm affine not implemented"

    cen = np.eye(H) - np.full((H, H), 1.0 / H)
    w2 = np.concatenate([wk.T @ cen, wq.T @ cen], axis=1).astype(np.float16)
    wu1 = np.ascontiguousarray(wu[:, :H].T).astype(np.float16)
    wu2 = np.ascontiguousarray((wu[:, H:] @ wm).T).astype(np.float16)

    maps_a = []
    for c in range(NC):
        maps_a.append(dict(hT=prep["hT"][c], w2=w2, nm=prep["nmt"][c]))
    return maps_a, wu1, wu2


def _make_tables(prep, res_a):
    """k-table [NC*SLAB, EW] f32-view (fp16 content) + per-core q tables."""
    h = prep["h"]

    def untile(t):  # [PG, NGT*EW] tile -> [RPAD, EW] rows (node g*128+p)
        return t.reshape(PG, NGT, EW).transpose(1, 0, 2).reshape(RPAD, EW)

    ktab16 = np.zeros((NC, SLAB, 2 * EW), np.float16)
    for c in range(NC):
        kl = untile(res_a[c]["kl"])              # [RPAD, EW] f16
        ktab16[c, :, 0:EW] = kl
        ktab16[c, :BLK, EW:EW + H] = h[c * BLK:(c + 1) * BLK].astype(np.float16)
        ktab16[c, BLK:, H] = LMNEG               # poison rows
    ktab = ktab16.reshape(NC * SLAB, 2 * EW).view(np.float32)
    qtabs = []
    for c in range(NC):
        qq = untile(res_a[c]["qq"])              # [RPAD, EW] f16
        q16 = np.zeros((SLAB, 2 * EW), np.float16)
        q16[:, 0:EW] = qq
        qtabs.append(q16.view(np.float32))
    return ktab, qtabs


def kernel(**inputs):
    from concourse.bass_utils import run_bass_kernel_spmd

    prep = _prep(inputs)
    maps_a, wu1, wu2 = _make_maps(inputs, prep)

    nc_a = _build_a()
    res_a = run_bass_kernel_spmd(nc_a, maps_a, core_ids=list(range(NC))).results

    ktab, qtabs = _make_tables(prep, res_a)

    nc_b = _build_b(prep["TC"], prep["sgs"], prep["kpieces"],
                    prep["spieces"], prep["qpieces"])
    maps_b = []
    for c in range(NC):
        m = dict(prep["in_b"][c])
        m["ktab"] = ktab
        m["qtab"] = qtabs[c]
        m["wu1"] = wu1
        m["wu2"] = wu2
        m["hpT"] = prep["hpT"][c]
        m["hp2"] = prep["hp2"][c]
        maps_b.append(m)
    res_b = run_bass_kernel_spmd(nc_b, maps_b, core_ids=list(range(NC))).results

    out = np.empty((N, H), np.float32)
    for c in range(NC):
        ob = res_b[c]["out"].astype(np.float32)  # [RPAD, H], row = node p*NGT+j
        out[c * BLK:(c + 1) * BLK] = ob[:BLK]
    return out


# revision 39
# speedup vs baseline: 1.2767x; 1.1179x over previous
"""AttentionMPLayer on 8 Trainium2 NeuronCores (Bass/Tile).

Sharding: nodes in 8 contiguous blocks (12500/core); edges routed to the core
owning their src node.  Within a core edges are packed DENSELY (128 per
column, no per-row alignment), sorted by dst-core so each dma_gather call
reads one 12544-row table slab with int16 indices.

Per edge the device gathers a 256B table row [k48|lm|pad|h48|pad] (fp16
content, gathered as f32x64) and a 256B q row [q48|1|pad], computes
score = q~.(k~ + 0.1 ef), w = exp(score), and dma_scatter_adds [w*h | w]
(49 f32) into a per-src-node accumulator.  A tail phase normalizes
(agg = num/den), applies the output head (Wu1/Wu2 with Wm folded), leaky
relu, and a batched LayerNorm (rsqrt via exp(-0.5 ln)).

Kernel A computes k~ = LN(h@Wk.T) and q~ = LN(h@Wq.T) with the mean
subtraction folded into host-transformed weights W.T @ (I - J/48), variance
via fused multiply-reduce, and rstd via exp(-0.5 ln(var+eps)).
"""
import numpy as np

N, E, H, NC = 100000, 1600000, 48, 8
BLK = N // NC            # 12500 nodes per core
PG = 128
NGT = 98                 # tail groups (12544 = 128*98)
RPAD = PG * NGT          # 12544
SLAB = RPAD              # k-table rows per core slab
POISON = BLK             # first poison row within a slab
DUMP = RPAD - 1          # accumulator dump row for pad edges
EW = 64                  # f32 words per table row (= 128 fp16)
WW = 49                  # scatter payload f32 words [w*h(48) | w]
SGC = 20                 # columns (x128 edges) per supergroup
EPS_LN = 1e-5
EPS_DEN = 1e-30
LMNEG = -30000.0

_build_cache = {}


# ---------------------------------------------------------------- host routing
GCH = 8   # max columns (x128 descriptors) per gather/scatter call


def _chunk(lo, hi, bounds):
    """Split [lo,hi) at `bounds` and into <=GCH-col chunks."""
    cuts = sorted({lo, hi} | {b for b in bounds if lo < b < hi})
    out = []
    for a, b in zip(cuts[:-1], cuts[1:]):
        x = a
        while x < b:
            out.append((x, min(x + GCH, b)))
            x = min(x + GCH, b)
    return out


def _plan(src, dst):
    """Dense layout in (dst-core, occurrence-layer) cells, shared schedule.

    Within a cell every edge has a distinct src (occurrence index within
    (src, dst-core) is constant), so scatter-add calls confined to one cell
    have unique indices.  Cells are padded to 128-edge column boundaries.
    """
    percore = []
    cellcnt = {}   # (c, dc, k) -> count
    maxk = np.zeros(NC, np.int64)
    for c in range(NC):
        m = np.nonzero((src >= c * BLK) & (src < (c + 1) * BLK))[0]
        s_loc = src[m] - c * BLK
        dc = dst[m] // BLK
        order = np.lexsort((s_loc, dc))
        m = m[order]
        s_loc = s_loc[order]
        dc = dc[order]
        # occurrence index within (dc, src) runs (sorted, so runs contiguous)
        key = dc * BLK + s_loc
        first = np.searchsorted(key, key, side="left")
        k = np.arange(len(m)) - first
        order2 = np.lexsort((s_loc, k, dc))
        m = m[order2]
        k = k[order2]
        dc = dc[order2]
        percore.append((m, dc, k))
        for dcv in range(NC):
            sel = dc == dcv
            if sel.any():
                kk = k[sel]
                maxk[dcv] = max(maxk[dcv], kk.max() + 1)
                bc = np.bincount(kk)
                for kv, n in enumerate(bc):
                    if n:
                        cellcnt[(c, dcv, kv)] = int(n)
    # shared cell column counts
    cells = []          # ordered (dc, k, cols)
    for dcv in range(NC):
        for kv in range(int(maxk[dcv])):
            n = max(cellcnt.get((c, dcv, kv), 0) for c in range(NC))
            if n:
                cells.append((dcv, kv, (n + PG - 1) // PG))
    CO = {}
    off = 0
    seg_lo = {}
    seg_hi = {}
    for (dcv, kv, cols) in cells:
        CO[(dcv, kv)] = off
        seg_lo.setdefault(dcv, off)
        seg_hi[dcv] = off + cols
        off += cols
    TC = off
    cell_bounds = sorted(CO.values()) + [TC]

    sgs = []
    c0 = 0
    while c0 < TC:
        sgs.append((c0, min(SGC, TC - c0)))
        c0 += SGC
    kpieces, spieces, qpieces = [], [], []
    for (c0, ncs) in sgs:
        kp = []
        for dcv in range(NC):
            if dcv not in seg_lo:
                continue
            lo, hi = max(c0, seg_lo[dcv]), min(c0 + ncs, seg_hi[dcv])
            if lo < hi:
                kp.extend((dcv, a - c0, b - c0) for (a, b) in
                          _chunk(lo, hi, cell_bounds))
        kpieces.append(kp)
        spieces.append([(a - c0, b - c0) for (a, b) in
                        _chunk(c0, c0 + ncs, cell_bounds)])
        qpieces.append([(a - c0, b - c0) for (a, b) in
                        _chunk(c0, c0 + ncs, [])])
    return percore, cells, CO, TC, sgs, kpieces, spieces, qpieces


def _prep(inputs):
    h = np.asarray(inputs["h"], np.float32)
    ei = np.asarray(inputs["edge_index"])
    ea = np.asarray(inputs["edge_attr"], np.float32)
    nm = np.asarray(inputs["node_mult"], np.float32)
    src = ei[0].astype(np.int64)
    dst = ei[1].astype(np.int64)
    percore, cells, CO, TC, sgs, kpieces, spieces, qpieces = _plan(src, dst)

    def wrap16(flat, vals, pos):
        # idx for position i lives at [i % 16 + 16*q7core, i // 16]
        r, cc = pos % 16, pos // 16
        for q7 in range(8):
            flat[16 * q7 + r, cc] = vals

    in_b = []
    for c in range(NC):
        m, dc, k = percore[c]
        kix = np.full((PG, TC * 8), POISON, np.int16)
        six = np.full((PG, TC * 8), DUMP, np.int16)
        qix = np.zeros((PG, TC * 8), np.int16)
        ef2 = np.zeros((PG, TC, H), np.float16)
        pos = np.empty(len(m), np.int64)
        for (dcv, kv, cols) in cells:
            sel = np.nonzero((dc == dcv) & (k == kv))[0]
            pos[sel] = CO[(dcv, kv)] * PG + np.arange(len(sel))
        wrap16(kix, (dst[m] % BLK).astype(np.int16), pos)
        wrap16(qix, (src[m] - c * BLK).astype(np.int16), pos)
        wrap16(six, (src[m] - c * BLK).astype(np.int16), pos)
        ef2[pos % PG, pos // PG, 0:H] = (0.1 * ea[m]).astype(np.float16)
        # one combined idx stream per supergroup: [kix | qix | six] blocks
        idx3 = np.empty((PG, TC * 24), np.int16)
        for (c0, ncs) in sgs:
            blk = idx3[:, c0 * 24:(c0 + ncs) * 24]
            blk[:, 0:ncs * 8] = kix[:, c0 * 8:(c0 + ncs) * 8]
            blk[:, ncs * 8:ncs * 16] = qix[:, c0 * 8:(c0 + ncs) * 8]
            blk[:, ncs * 16:ncs * 24] = six[:, c0 * 8:(c0 + ncs) * 8]
        in_b.append(dict(idx3=idx3, ef2=ef2.reshape(PG, TC * H)))

    # tail inputs: node order on tile = row p*NGT + j
    hp16 = np.zeros((NC, RPAD, H), np.float16)
    for c in range(NC):
        hp16[c, :BLK] = h[c * BLK:(c + 1) * BLK].astype(np.float16)
    hp2 = np.ascontiguousarray(hp16.reshape(NC, PG, NGT * H))
    hpT = np.zeros((NC, H, RPAD), np.float16)
    for c in range(NC):
        # hpT[:, j*128 + p] = h[p*NGT + j]
        v = hp16[c].reshape(PG, NGT, H)          # [p, j, e]
        hpT[c] = np.ascontiguousarray(v.transpose(2, 1, 0).reshape(H, RPAD))

    # kernel A inputs
    hT = np.zeros((NC, H, RPAD), np.float16)
    nmt = np.ones((NC, PG, NGT), np.float32)
    for c in range(NC):
        blk = h[c * BLK:(c + 1) * BLK].astype(np.float16)  # [BLK, H]
        hT[c, :, :BLK] = blk.T
        tmp = np.ones(RPAD, np.float32)
        tmp[:BLK] = nm[c * BLK:(c + 1) * BLK]
        nmt[c] = tmp.reshape(NGT, PG).T  # A-tile [p, g] = node g*128+p
    return dict(h=h, TC=TC, sgs=sgs, kpieces=kpieces, spieces=spieces,
                qpieces=qpieces, in_b=in_b,
                hp2=hp2, hpT=hpT, hT=hT, nmt=nmt)


# ------------------------------------------------------------------- kernel A
def _build_a():
    if "A" in _build_cache:
        return _build_cache["A"]
    import concourse.bacc as bacc
    import concourse.tile as tile
    import concourse.mybir as mybir

    nc = bacc.Bacc("TRN2", target_bir_lowering=False, debug=False,
                   num_devices=NC)
    f32 = mybir.dt.float32
    f16 = mybir.dt.float16
    t_hT = nc.dram_tensor("hT", [H, RPAD], f16, kind="ExternalInput").ap()
    t_w2 = nc.dram_tensor("w2", [H, 2 * H], f16, kind="ExternalInput").ap()
    t_nm = nc.dram_tensor("nm", [PG, NGT], f32, kind="ExternalInput").ap()
    # tile layout [p, g*EW+e] = node g*128+p; host transposes
    t_kl = nc.dram_tensor("kl", [PG, NGT * EW], f16, kind="ExternalOutput").ap()
    t_qq = nc.dram_tensor("qq", [PG, NGT * EW], f16, kind="ExternalOutput").ap()

    mult = mybir.AluOpType.mult
    add = mybir.AluOpType.add
    AXX = mybir.AxisListType.X
    EXP = mybir.ActivationFunctionType.Exp
    LN_F = mybir.ActivationFunctionType.Ln
    COPY = mybir.ActivationFunctionType.Copy

    from concourse import library_config
    with tile.TileContext(nc) as tc, nc.allow_low_precision(reason="fp16 ln"):
        with tc.tile_pool(name="const", bufs=1) as cpool, \
             tc.tile_pool(name="work", bufs=6) as wpool, \
             tc.tile_pool(name="ps", bufs=6, space="PSUM") as ppool:
            nc.gpsimd.load_library(library_config.standard)
            w2_s = cpool.tile([H, 2 * H], f16)
            nc.sync.dma_start(out=w2_s[:], in_=t_w2)
            hT_s = cpool.tile([H, RPAD], f16)
            nc.sync.dma_start(out=hT_s[:], in_=t_hT)
            nm_s = cpool.tile([PG, NGT], f32)
            nc.sync.dma_start(out=nm_s[:], in_=t_nm)
            xc_s = cpool.tile([PG, NGT * 2 * H], f16)
            varT = cpool.tile([PG, 2 * NGT], f32)
            kl_s = cpool.tile([PG, NGT * EW], f16)
            qq_s = cpool.tile([PG, NGT * EW], f16)
            # pad columns [49:64) are stored to DRAM; zero them once
            nc.vector.memset(
                kl_s[:].rearrange("p (g e) -> p g e", g=NGT)[:, :, H + 1:EW], 0.0)
            nc.vector.memset(
                qq_s[:].rearrange("p (g e) -> p g e", g=NGT)[:, :, H + 1:EW], 0.0)

            for g in range(NGT):
                ps = ppool.tile([PG, 2 * H], f32, tag="ps")
                nc.tensor.matmul(out=ps[:], lhsT=hT_s[:, g * PG:(g + 1) * PG],
                                 rhs=w2_s[:], start=True, stop=True)
                xc = xc_s[:, g * 2 * H:(g + 1) * 2 * H]
                nc.scalar.activation(out=xc, in_=ps[:], func=COPY)
                sq = wpool.tile([PG, 2 * H], f16, tag="sq")
                nc.gpsimd.tensor_tensor(out=sq[:], in0=xc, in1=xc, op=mult)
                nc.vector.tensor_reduce(
                    out=varT[:, 2 * g:2 * g + 2].unsqueeze(2),
                    in_=sq[:].rearrange("p (s e) -> p s e", s=2),
                    axis=AXX, op=add)
            # rstd = exp(-0.5 * ln(sumsq/H + eps))
            eps_s = cpool.tile([PG, 1], f32)
            nc.vector.memset(eps_s[:], EPS_LN)
            lv = cpool.tile([PG, 2 * NGT], f32)
            nc.scalar.activation(out=lv[:], in_=varT[:], func=LN_F,
                                 bias=eps_s[:], scale=1.0 / H)
            rstd = cpool.tile([PG, 2 * NGT], f32)
            nc.scalar.activation(out=rstd[:], in_=lv[:], func=EXP, scale=-0.5)
            # lm = ln(max(nm, 1))
            lmx = cpool.tile([PG, NGT], f32)
            nc.vector.tensor_scalar_max(lmx[:], nm_s[:], 1.0)
            lm = cpool.tile([PG, NGT], f32)
            nc.scalar.activation(out=lm[:], in_=lmx[:], func=LN_F)
            kl3 = kl_s[:].rearrange("p (g e) -> p g e", g=NGT)
            qq3 = qq_s[:].rearrange("p (g e) -> p g e", g=NGT)
            nc.vector.tensor_copy(kl3[:, :, H:H + 1], lm[:].unsqueeze(2))
            nc.vector.memset(qq3[:, :, H:H + 1], 1.0)
            for g in range(NGT):
                xc = xc_s[:, g * 2 * H:(g + 1) * 2 * H]
                nc.vector.tensor_scalar_mul(
                    kl_s[:, g * EW:g * EW + H], xc[:, 0:H],
                    rstd[:, 2 * g:2 * g + 1])
                nc.vector.tensor_scalar_mul(
                    qq_s[:, g * EW:g * EW + H], xc[:, H:2 * H],
                    rstd[:, 2 * g + 1:2 * g + 2])
            nc.sync.dma_start(out=t_kl, in_=kl_s[:])
            nc.sync.dma_start(out=t_qq, in_=qq_s[:])
    nc.compile()
    _build_cache["A"] = nc
    return nc


# ------------------------------------------------------------------- kernel B
def _build_b(TC, sgs, kpieces, spieces, qpieces):
    key = ("B", TC, tuple(sgs), str(kpieces), str(spieces), str(qpieces))
    if key in _build_cache:
        return _build_cache[key]
    import concourse.bacc as bacc
    import concourse.tile as tile
    import concourse.mybir as mybir
    from concourse.masks import make_identity
    from concourse import library_config

    nc = bacc.Bacc("TRN2", target_bir_lowering=False, debug=False,
                   num_devices=NC)
    f32 = mybir.dt.float32
    f16 = mybir.dt.float16
    bf16 = mybir.dt.bfloat16
    i16 = mybir.dt.int16
    t_ktab = nc.dram_tensor("ktab", [NC * SLAB, EW], f32,
                            kind="ExternalInput").ap()
    t_qtab = nc.dram_tensor("qtab", [SLAB, EW], f32, kind="ExternalInput").ap()
    t_ef2 = nc.dram_tensor("ef2", [PG, TC * H], f16,
                           kind="ExternalInput").ap()
    t_idx3 = nc.dram_tensor("idx3", [PG, TC * 24], i16,
                            kind="ExternalInput").ap()
    t_wu1 = nc.dram_tensor("wu1", [H, H], f16, kind="ExternalInput").ap()
    t_wu2 = nc.dram_tensor("wu2", [H, H], f16, kind="ExternalInput").ap()
    t_hpT = nc.dram_tensor("hpT", [H, RPAD], f16, kind="ExternalInput").ap()
    t_hp2 = nc.dram_tensor("hp2", [PG, NGT * H], f16,
                           kind="ExternalInput").ap()
    t_out = nc.dram_tensor("out", [RPAD, H], f16, kind="ExternalOutput").ap()
    t_acc = nc.dram_tensor("acc", [RPAD, 2 * EW], bf16, kind="Internal").ap()

    mult = mybir.AluOpType.mult
    add = mybir.AluOpType.add
    sub = mybir.AluOpType.subtract
    amax = mybir.AluOpType.max
    AXX = mybir.AxisListType.X
    EXP = mybir.ActivationFunctionType.Exp
    LN_F = mybir.ActivationFunctionType.Ln
    COPY = mybir.ActivationFunctionType.Copy

    with tile.TileContext(nc) as tc, nc.allow_low_precision(reason="fp16"):
        with tc.tile_pool(name="const", bufs=1) as cpool, \
             tc.tile_pool(name="idx", bufs=5) as ipool, \
             tc.tile_pool(name="gat", bufs=5) as gpool, \
             tc.tile_pool(name="wrk", bufs=5) as wpool, \
             tc.tile_pool(name="tl", bufs=6) as tpool, \
             tc.tile_pool(name="ps", bufs=4, space="PSUM") as ppool:
            nc.gpsimd.load_library(library_config.mlp)
            wu1_s = cpool.tile([H, H], f16)
            nc.sync.dma_start(out=wu1_s[:], in_=t_wu1)
            wu2_s = cpool.tile([H, H], f16)
            nc.sync.dma_start(out=wu2_s[:], in_=t_wu2)
            ident = cpool.tile([PG, PG], f16)
            make_identity(nc, ident)
            NZ = NGT * 2 * EW // 7
            z_s = cpool.tile([PG, NZ], bf16)
            nc.vector.memset(z_s[:], 0.0)
            accv = t_acc.rearrange("(p q x) e -> p q (x e)", p=PG, q=7)
            for qq in range(7):
                nc.scalar.dma_start(out=accv[:, qq, :], in_=z_s[:])

            for si, (c0, ncs) in enumerate(sgs):
                idx_t = ipool.tile([PG, ncs * 24], i16, tag="idx")
                nc.sync.dma_start(out=idx_t[:],
                                  in_=t_idx3[:, c0 * 24:(c0 + ncs) * 24])
                kix_t = idx_t[:, 0:ncs * 8]
                qix_t = idx_t[:, ncs * 8:ncs * 16]
                six_t = idx_t[:, ncs * 16:ncs * 24]
                ef_t = wpool.tile([PG, ncs * H], f16, tag="ef")
                nc.sync.dma_start(out=ef_t[:],
                                  in_=t_ef2[:, c0 * H:(c0 + ncs) * H])
                g_k = gpool.tile([PG, ncs * EW], f32, tag="gk")
                for (cp, r0, r1) in kpieces[si]:
                    nc.gpsimd.dma_gather(
                        out_ap=g_k[:, r0 * EW:r1 * EW].rearrange(
                            "p (x e) -> p x e", e=EW),
                        in_ap=t_ktab[cp * SLAB:(cp + 1) * SLAB, :],
                        idxs_ap=kix_t[:, r0 * 8:r1 * 8],
                        num_idxs=(r1 - r0) * PG,
                        num_idxs_reg=(r1 - r0) * PG,
                        elem_size=EW)
                g_q = gpool.tile([PG, ncs * EW], f32, tag="gq")
                for (r0, r1) in qpieces[si]:
                    nc.gpsimd.dma_gather(
                        out_ap=g_q[:, r0 * EW:r1 * EW].rearrange(
                            "p (x e) -> p x e", e=EW),
                        in_ap=t_qtab,
                        idxs_ap=qix_t[:, r0 * 8:r1 * 8],
                        num_idxs=(r1 - r0) * PG,
                        num_idxs_reg=(r1 - r0) * PG,
                        elem_size=EW)
                gk6 = g_k[:].bitcast(f16).rearrange("p (x e) -> p x e", e=2 * EW)
                gq6 = g_q[:].bitcast(f16).rearrange("p (x e) -> p x e", e=2 * EW)
                ef3 = ef_t[:].rearrange("p (x e) -> p x e", e=H)
                kef = wpool.tile([PG, ncs * H], f16, tag="kef")
                kef3 = kef[:].rearrange("p (x e) -> p x e", e=H)
                nc.vector.tensor_tensor(out=kef3, in0=gk6[:, :, 0:H],
                                        in1=ef3, op=add)
                prod = wpool.tile([PG, ncs * H], f16, tag="prod")
                prod3 = prod[:].rearrange("p (x e) -> p x e", e=H)
                nc.vector.tensor_tensor(out=prod3, in0=kef3,
                                        in1=gq6[:, :, 0:H], op=mult)
                score = wpool.tile([PG, ncs], f32, tag="score")
                nc.vector.tensor_reduce(out=score[:].unsqueeze(2), in_=prod3,
                                        axis=AXX, op=add)
                nc.vector.tensor_tensor(out=score[:].unsqueeze(2),
                                        in0=score[:].unsqueeze(2),
                                        in1=gk6[:, :, H:H + 1], op=add)
                esc48 = wpool.tile([PG, ncs * H], bf16, tag="esc48")
                nc.scalar.activation(
                    out=esc48[:].rearrange("p (x e) -> p x e", e=H),
                    in_=score[:].unsqueeze(2).to_broadcast([PG, ncs, H]),
                    func=EXP)
                e48v = esc48[:].rearrange("p (x e) -> p x e", e=H)
                w_t = wpool.tile([PG, ncs * WW], bf16, tag="w")
                w3 = w_t[:].rearrange("p (x e) -> p x e", e=WW)
                nc.vector.tensor_tensor(
                    out=w3[:, :, 0:H], in0=gk6[:, :, EW:EW + H],
                    in1=e48v, op=mult)
                nc.vector.tensor_copy(w3[:, :, H:WW], e48v[:, :, 0:1])
                for (r0, r1) in spieces[si]:
                    nc.gpsimd.dma_scatter_add(
                        out_ap=t_acc[:, 0:WW],
                        in_ap=w3[:, r0:r1, :],
                        idxs_ap=six_t[:, r0 * 8:r1 * 8],
                        num_idxs=(r1 - r0) * PG,
                        num_idxs_reg=(r1 - r0) * PG,
                        elem_size=WW,
                        elem_step=2 * EW)

            # ------------------------------------------------------- tail
            nc.gpsimd.load_library(library_config.standard)
            acc_t = cpool.tile([PG, NGT * 2 * EW], bf16)
            nc.sync.dma_start(
                out=acc_t[:].rearrange("p (x e) -> p x e", e=2 * EW),
                in_=t_acc.rearrange("(p x) e -> p x e", p=PG))
            hpT_s = cpool.tile([H, RPAD], f16)
            nc.sync.dma_start(out=hpT_s[:], in_=t_hpT)
            hp2_s = cpool.tile([PG, NGT * H], f16)
            nc.sync.dma_start(out=hp2_s[:], in_=t_hp2)
            acc3 = acc_t[:].rearrange("p (x e) -> p x e", e=2 * EW)
            den = cpool.tile([PG, NGT], f32)
            nc.vector.tensor_scalar_add(den[:].unsqueeze(2),
                                        acc3[:, :, H:H + 1], EPS_DEN)
            rin = cpool.tile([PG, NGT], f32)
            nc.vector.reciprocal(out=rin[:], in_=den[:])
            r16 = cpool.tile([PG, NGT * H], f16)
            sumT = cpool.tile([PG, NGT], f32)
            varT = cpool.tile([PG, NGT], f32)
            for j in range(NGT):
                agg16 = tpool.tile([PG, H], f16, tag="agg16")
                nc.vector.tensor_scalar_mul(
                    agg16[:], acc_t[:, j * 2 * EW:j * 2 * EW + H],
                    rin[:, j:j + 1])
                aggT = ppool.tile([H, PG], f16, tag="aggT")
                nc.tensor.transpose(out=aggT[:], in_=agg16[:],
                                    identity=ident[:])
                aggTs = tpool.tile([H, PG], f16, tag="aggTs")
                nc.scalar.activation(out=aggTs[:], in_=aggT[:], func=COPY)
                zp = ppool.tile([PG, H], f32, tag="zp")
                nc.tensor.matmul(out=zp[:], lhsT=hpT_s[:, j * PG:(j + 1) * PG],
                                 rhs=wu1_s[:], start=True, stop=False)
                nc.tensor.matmul(out=zp[:], lhsT=aggTs[:], rhs=wu2_s[:],
                                 start=False, stop=True)
                zs = tpool.tile([PG, H], f16, tag="zs")
                nc.scalar.activation(out=zs[:], in_=zp[:], func=COPY,
                                     scale=0.01)
                z16 = tpool.tile([PG, H], f16, tag="z16")
                nc.vector.tensor_tensor(out=z16[:], in0=zp[:], in1=zs[:],
                                        op=amax)
                rj = r16[:, j * H:(j + 1) * H]
                nc.gpsimd.tensor_tensor(out=rj, in0=z16[:],
                                        in1=hp2_s[:, j * H:(j + 1) * H],
                                        op=add)
                dmy = tpool.tile([PG, H], f16, tag="dmy")
                nc.vector.tensor_scalar(
                    out=dmy[:], in0=rj, scalar1=1.0, scalar2=0.0,
                    op0=mult, op1=add, accum_out=sumT[:, j:j + 1])
                sq = tpool.tile([PG, H], f16, tag="sqt")
                nc.gpsimd.tensor_tensor(out=sq[:], in0=rj, in1=rj, op=mult)
                nc.vector.tensor_scalar(
                    out=dmy[:], in0=sq[:], scalar1=1.0, scalar2=0.0,
                    op0=mult, op1=add, accum_out=varT[:, j:j + 1])
            mean = cpool.tile([PG, NGT], f32)
            nc.vector.tensor_scalar_mul(mean[:], sumT[:], 1.0 / H)
            m2 = cpool.tile([PG, NGT], f32)
            nc.vector.tensor_tensor(out=m2[:], in0=mean[:], in1=mean[:],
                                    op=mult)
            var = cpool.tile([PG, NGT], f32)
            nc.vector.tensor_scalar_mul(var[:], varT[:], 1.0 / H)
            nc.vector.tensor_tensor(out=var[:], in0=var[:], in1=m2[:], op=sub)
            eps_s = cpool.tile([PG, 1], f32)
            nc.vector.memset(eps_s[:], EPS_LN)
            lv = cpool.tile([PG, NGT], f32)
            nc.scalar.activation(out=lv[:], in_=var[:], func=LN_F,
                                 bias=eps_s[:], scale=1.0)
            rstd = cpool.tile([PG, NGT], f32)
            nc.scalar.activation(out=rstd[:], in_=lv[:], func=EXP, scale=-0.5)
            nmr = cpool.tile([PG, NGT], f32)
            nc.vector.tensor_tensor(out=nmr[:], in0=mean[:], in1=rstd[:],
                                    op=mult)
            nc.vector.tensor_scalar_mul(nmr[:], nmr[:], -1.0)
            on = cpool.tile([PG, NGT * H], f16)
            for j in range(NGT):
                nc.vector.tensor_scalar(
                    out=on[:, j * H:(j + 1) * H], in0=r16[:, j * H:(j + 1) * H],
                    scalar1=rstd[:, j:j + 1], scalar2=nmr[:, j:j + 1],
                    op0=mult, op1=add)
            nc.sync.dma_start(
                out=t_out.rearrange("(p x) e -> p x e", p=PG),
                in_=on[:].rearrange("p (x e) -> p x e", e=H))
    nc.compile()
    _build_cache[key] = nc
    return nc


# -------------------------------------------------------------------- driver
def _make_maps(inputs, prep):
    h = prep["h"]
    wq = np.asarray(inputs["Wq"], np.float64)
    wk = np.asarray(inputs["Wk"], np.float64)
    wm = np.asarray(inputs["Wm"], np.float64)
    wu = np.asarray(inputs["Wu"], np.float64)
    gq = np.asarray(inputs["gq"], np.float32)
    bq = np.asarray(inputs["bq"], np.float32)
    gk = np.asarray(inputs["gk"], np.float32)
    bk = np.asarray(inputs["bk"], np.float32)
    go = np.asarray(inputs["go"], np.float32)
    bo = np.asarray(inputs["bo"], np.float32)
    triv = (np.all(gq == 1) and np.all(gk == 1) and np.all(go == 1)
            and np.all(bq == 0) and np.all(bk == 0) and np.all(bo == 0))
    assert triv, "non-trivial layernorm affine not implemented"

    cen = np.eye(H) - np.full((H, H), 1.0 / H)
    w2 = np.concatenate([wk.T @ cen, wq.T @ cen], axis=1).astype(np.float16)
    wu1 = np.ascontiguousarray(wu[:, :H].T).astype(np.float16)
    wu2 = np.ascontiguousarray((wu[:, H:] @ wm).T).astype(np.float16)

    maps_a = []
    for c in range(NC):
        maps_a.append(dict(hT=prep["hT"][c], w2=w2, nm=prep["nmt"][c]))
    return maps_a, wu1, wu2


def _make_tables(prep, res_a):
    """k-table [NC*SLAB, EW] f32-view (fp16 content) + per-core q tables."""
    h = prep["h"]

    def untile(t):  # [PG, NGT*EW] tile -> [RPAD, EW] rows (node g*128+p)
        return t.reshape(PG, NGT, EW).transpose(1, 0, 2).reshape(RPAD, EW)

    ktab16 = np.zeros((NC, SLAB, 2 * EW), np.float16)
    for c in range(NC):
        kl = untile(res_a[c]["kl"])              # [RPAD, EW] f16
        ktab16[c, :, 0:EW] = kl
        ktab16[c, :BLK, EW:EW + H] = h[c * BLK:(c + 1) * BLK].astype(np.float16)
        ktab16[c, BLK:, H] = LMNEG               # poison rows
    ktab = ktab16.reshape(NC * SLAB, 2 * EW).view(np.float32)
    qtabs = []
    for c in range(NC):
        qq = untile(res_a[c]["qq"])              # [RPAD, EW] f16
        q16 = np.zeros((SLAB, 2 * EW), np.float16)
        q16[:, 0:EW] = qq
        qtabs.append(q16.view(np.float32))
    return ktab, qtabs


def kernel(**inputs):
    from concourse.bass_utils import run_bass_kernel_spmd

    prep = _prep(inputs)
    maps_a, wu1, wu2 = _make_maps(inputs, prep)

    nc_a = _build_a()
    res_a = run_bass_kernel_spmd(nc_a, maps_a, core_ids=list(range(NC))).results

    ktab, qtabs = _make_tables(prep, res_a)

    nc_b = _build_b(prep["TC"], prep["sgs"], prep["kpieces"],
                    prep["spieces"], prep["qpieces"])
    maps_b = []
    for c in range(NC):
        m = dict(prep["in_b"][c])
        m["ktab"] = ktab
        m["qtab"] = qtabs[c]
        m["wu1"] = wu1
        m["wu2"] = wu2
        m["hpT"] = prep["hpT"][c]
        m["hp2"] = prep["hp2"][c]
        maps_b.append(m)
    res_b = run_bass_kernel_spmd(nc_b, maps_b, core_ids=list(range(NC))).results

    out = np.empty((N, H), np.float32)
    for c in range(NC):
        ob = res_b[c]["out"].astype(np.float32)  # [RPAD, H], row = node p*NGT+j
        out[c * BLK:(c + 1) * BLK] = ob[:BLK]
    return out


# revision 43
# speedup vs baseline: 1.3253x; 1.0381x over previous
"""AttentionMPLayer on 8 Trainium2 NeuronCores (Bass/Tile).

Sharding: nodes in 8 contiguous blocks (12500/core); edges routed to the core
owning their src node.  Within a core edges are packed DENSELY (128 per
column, no per-row alignment), sorted by dst-core so each dma_gather call
reads one 12544-row table slab with int16 indices.

Per edge the device gathers a 256B table row [k48|lm|pad|h48|pad] (fp16
content, gathered as f32x64) and a 256B q row [q48|1|pad], computes
score = q~.(k~ + 0.1 ef), w = exp(score), and dma_scatter_adds [w*h | w]
(49 f32) into a per-src-node accumulator.  A tail phase normalizes
(agg = num/den), applies the output head (Wu1/Wu2 with Wm folded), leaky
relu, and a batched LayerNorm (rsqrt via exp(-0.5 ln)).

Kernel A computes k~ = LN(h@Wk.T) and q~ = LN(h@Wq.T) with the mean
subtraction folded into host-transformed weights W.T @ (I - J/48), variance
via fused multiply-reduce, and rstd via exp(-0.5 ln(var+eps)).
"""
import numpy as np

N, E, H, NC = 100000, 1600000, 48, 8
BLK = N // NC            # 12500 nodes per core
PG = 128
NGT = 98                 # tail groups (12544 = 128*98)
RPAD = PG * NGT          # 12544
SLAB = RPAD              # k-table rows per core slab
POISON = BLK             # first poison row within a slab
DUMP = RPAD - 1          # accumulator dump row for pad edges
EW = 64                  # f32 words per table row (= 128 fp16)
WW = 49                  # scatter payload f32 words [w*h(48) | w]
SGC = 20                 # columns (x128 edges) per supergroup
EPS_LN = 1e-5
EPS_DEN = 1e-30
LMNEG = -30000.0

_build_cache = {}


# ---------------------------------------------------------------- host routing
GCH = 8   # max columns (x128 descriptors) per gather/scatter call


def _chunk(lo, hi, bounds):
    """Split [lo,hi) at `bounds` and into <=GCH-col chunks."""
    cuts = sorted({lo, hi} | {b for b in bounds if lo < b < hi})
    out = []
    for a, b in zip(cuts[:-1], cuts[1:]):
        x = a
        while x < b:
            out.append((x, min(x + GCH, b)))
            x = min(x + GCH, b)
    return out


def _plan(src, dst):
    """Dense layout in (dst-core, occurrence-layer) cells, shared schedule.

    Within a cell every edge has a distinct src (occurrence index within
    (src, dst-core) is constant), so scatter-add calls confined to one cell
    have unique indices.  Cells are padded to 128-edge column boundaries.
    """
    percore = []
    cellcnt = {}   # (c, dc, k) -> count
    maxk = np.zeros(NC, np.int64)
    for c in range(NC):
        m = np.nonzero((src >= c * BLK) & (src < (c + 1) * BLK))[0]
        s_loc = src[m] - c * BLK
        dc = dst[m] // BLK
        order = np.lexsort((s_loc, dc))
        m = m[order]
        s_loc = s_loc[order]
        dc = dc[order]
        # occurrence index within (dc, src) runs (sorted, so runs contiguous)
        key = dc * BLK + s_loc
        first = np.searchsorted(key, key, side="left")
        k = np.arange(len(m)) - first
        order2 = np.lexsort((s_loc, k, dc))
        m = m[order2]
        k = k[order2]
        dc = dc[order2]
        percore.append((m, dc, k))
        for dcv in range(NC):
            sel = dc == dcv
            if sel.any():
                kk = k[sel]
                maxk[dcv] = max(maxk[dcv], kk.max() + 1)
                bc = np.bincount(kk)
                for kv, n in enumerate(bc):
                    if n:
                        cellcnt[(c, dcv, kv)] = int(n)
    # shared cell column counts
    cells = []          # ordered (dc, k, cols)
    for dcv in range(NC):
        for kv in range(int(maxk[dcv])):
            n = max(cellcnt.get((c, dcv, kv), 0) for c in range(NC))
            if n:
                cells.append((dcv, kv, (n + PG - 1) // PG))
    CO = {}
    off = 0
    seg_lo = {}
    seg_hi = {}
    for (dcv, kv, cols) in cells:
        CO[(dcv, kv)] = off
        seg_lo.setdefault(dcv, off)
        seg_hi[dcv] = off + cols
        off += cols
    TC = off
    cell_bounds = sorted(CO.values()) + [TC]

    sgs = []
    c0 = 0
    while c0 < TC:
        sgs.append((c0, min(SGC, TC - c0)))
        c0 += SGC
    kpieces, spieces, qpieces = [], [], []
    for (c0, ncs) in sgs:
        kp = []
        for dcv in range(NC):
            if dcv not in seg_lo:
                continue
            lo, hi = max(c0, seg_lo[dcv]), min(c0 + ncs, seg_hi[dcv])
            if lo < hi:
                kp.extend((dcv, a - c0, b - c0) for (a, b) in
                          _chunk(lo, hi, cell_bounds))
        kpieces.append(kp)
        spieces.append([(a - c0, b - c0) for (a, b) in
                        _chunk(c0, c0 + ncs, cell_bounds)])
        qpieces.append([(a - c0, b - c0) for (a, b) in
                        _chunk(c0, c0 + ncs, [])])
    return percore, cells, CO, TC, sgs, kpieces, spieces, qpieces


def _prep(inputs):
    h = np.asarray(inputs["h"], np.float32)
    ei = np.asarray(inputs["edge_index"])
    ea = np.asarray(inputs["edge_attr"], np.float32)
    nm = np.asarray(inputs["node_mult"], np.float32)
    src = ei[0].astype(np.int64)
    dst = ei[1].astype(np.int64)
    percore, cells, CO, TC, sgs, kpieces, spieces, qpieces = _plan(src, dst)

    def wrap16(flat, vals, pos):
        # idx for position i lives at [i % 16 + 16*q7core, i // 16]
        r, cc = pos % 16, pos // 16
        for q7 in range(8):
            flat[16 * q7 + r, cc] = vals

    in_b = []
    for c in range(NC):
        m, dc, k = percore[c]
        kix = np.full((PG, TC * 8), POISON, np.int16)
        six = np.full((PG, TC * 8), DUMP, np.int16)
        qix = np.zeros((PG, TC * 8), np.int16)
        ef2 = np.zeros((PG, TC, H), np.float16)
        pos = np.empty(len(m), np.int64)
        for (dcv, kv, cols) in cells:
            sel = np.nonzero((dc == dcv) & (k == kv))[0]
            pos[sel] = CO[(dcv, kv)] * PG + np.arange(len(sel))
        wrap16(kix, (dst[m] % BLK).astype(np.int16), pos)
        wrap16(qix, (src[m] - c * BLK).astype(np.int16), pos)
        wrap16(six, (src[m] - c * BLK).astype(np.int16), pos)
        ef2[pos % PG, pos // PG, 0:H] = (0.1 * ea[m]).astype(np.float16)
        # one combined idx stream per supergroup: [kix | qix | six] blocks
        idx3 = np.empty((PG, TC * 24), np.int16)
        for (c0, ncs) in sgs:
            blk = idx3[:, c0 * 24:(c0 + ncs) * 24]
            blk[:, 0:ncs * 8] = kix[:, c0 * 8:(c0 + ncs) * 8]
            blk[:, ncs * 8:ncs * 16] = qix[:, c0 * 8:(c0 + ncs) * 8]
            blk[:, ncs * 16:ncs * 24] = six[:, c0 * 8:(c0 + ncs) * 8]
        in_b.append(dict(idx3=idx3, ef2=ef2.reshape(PG, TC * H)))

    # tail inputs: node order on tile = row p*NGT + j
    hp16 = np.zeros((NC, RPAD, H), np.float16)
    for c in range(NC):
        hp16[c, :BLK] = h[c * BLK:(c + 1) * BLK].astype(np.float16)
    hp2 = np.ascontiguousarray(hp16.reshape(NC, PG, NGT * H))
    hpT = np.zeros((NC, H, RPAD), np.float16)
    for c in range(NC):
        # hpT[:, j*128 + p] = h[p*NGT + j]
        v = hp16[c].reshape(PG, NGT, H)          # [p, j, e]
        hpT[c] = np.ascontiguousarray(v.transpose(2, 1, 0).reshape(H, RPAD))

    # kernel A inputs
    hT = np.zeros((NC, H, RPAD), np.float16)
    nmt = np.ones((NC, PG, NGT), np.float32)
    for c in range(NC):
        blk = h[c * BLK:(c + 1) * BLK].astype(np.float16)  # [BLK, H]
        hT[c, :, :BLK] = blk.T
        tmp = np.ones(RPAD, np.float32)
        tmp[:BLK] = nm[c * BLK:(c + 1) * BLK]
        nmt[c] = tmp.reshape(NGT, PG).T  # A-tile [p, g] = node g*128+p
    return dict(h=h, TC=TC, sgs=sgs, kpieces=kpieces, spieces=spieces,
                qpieces=qpieces, in_b=in_b,
                hp2=hp2, hpT=hpT, hT=hT, nmt=nmt)


# ------------------------------------------------------------------- kernel A
def _build_a():
    if "A" in _build_cache:
        return _build_cache["A"]
    import concourse.bacc as bacc
    import concourse.tile as tile
    import concourse.mybir as mybir

    nc = bacc.Bacc("TRN2", target_bir_lowering=False, debug=False,
                   num_devices=NC)
    f32 = mybir.dt.float32
    f16 = mybir.dt.float16
    t_hT = nc.dram_tensor("hT", [H, RPAD], f16, kind="ExternalInput").ap()
    t_w2 = nc.dram_tensor("w2", [H, 2 * H], f16, kind="ExternalInput").ap()
    t_nm = nc.dram_tensor("nm", [PG, NGT], f32, kind="ExternalInput").ap()
    # tile layout [p, g*EW+e] = node g*128+p; host transposes
    t_kl = nc.dram_tensor("kl", [PG, NGT * EW], f16, kind="ExternalOutput").ap()
    t_qq = nc.dram_tensor("qq", [PG, NGT * EW], f16, kind="ExternalOutput").ap()

    mult = mybir.AluOpType.mult
    add = mybir.AluOpType.add
    AXX = mybir.AxisListType.X
    EXP = mybir.ActivationFunctionType.Exp
    LN_F = mybir.ActivationFunctionType.Ln
    COPY = mybir.ActivationFunctionType.Copy

    from concourse import library_config
    with tile.TileContext(nc) as tc, nc.allow_low_precision(reason="fp16 ln"):
        with tc.tile_pool(name="const", bufs=1) as cpool, \
             tc.tile_pool(name="work", bufs=6) as wpool, \
             tc.tile_pool(name="ps", bufs=6, space="PSUM") as ppool:
            nc.gpsimd.load_library(library_config.standard)
            w2_s = cpool.tile([H, 2 * H], f16)
            nc.sync.dma_start(out=w2_s[:], in_=t_w2)
            hT_s = cpool.tile([H, RPAD], f16)
            nc.sync.dma_start(out=hT_s[:], in_=t_hT)
            nm_s = cpool.tile([PG, NGT], f32)
            nc.sync.dma_start(out=nm_s[:], in_=t_nm)
            xc_s = cpool.tile([PG, NGT * 2 * H], f16)
            varT = cpool.tile([PG, 2 * NGT], f32)
            kl_s = cpool.tile([PG, NGT * EW], f16)
            qq_s = cpool.tile([PG, NGT * EW], f16)
            # pad columns [49:64) are stored to DRAM; zero them once
            nc.vector.memset(
                kl_s[:].rearrange("p (g e) -> p g e", g=NGT)[:, :, H + 1:EW], 0.0)
            nc.vector.memset(
                qq_s[:].rearrange("p (g e) -> p g e", g=NGT)[:, :, H + 1:EW], 0.0)

            GB = 5
            blocks = []
            g0 = 0
            while g0 < NGT:
                blocks.append((g0, min(GB, NGT - g0)))
                g0 += GB
            for (g0, nb) in blocks:
                ps = ppool.tile([PG, GB * 2 * H], f32, tag="ps")
                for u in range(nb):
                    g = g0 + u
                    nc.tensor.matmul(out=ps[:, u * 2 * H:(u + 1) * 2 * H],
                                     lhsT=hT_s[:, g * PG:(g + 1) * PG],
                                     rhs=w2_s[:], start=True, stop=True)
                xc = xc_s[:, g0 * 2 * H:(g0 + nb) * 2 * H]
                nc.scalar.activation(out=xc, in_=ps[:, 0:nb * 2 * H], func=COPY)
            for (g0, nb) in blocks:
                xc = xc_s[:, g0 * 2 * H:(g0 + nb) * 2 * H]
                sq = wpool.tile([PG, GB * 2 * H], f16, tag="sq")
                nc.gpsimd.tensor_tensor(out=sq[:, 0:nb * 2 * H], in0=xc,
                                        in1=xc, op=mult)
                nc.vector.tensor_reduce(
                    out=varT[:, 2 * g0:2 * (g0 + nb)].unsqueeze(2),
                    in_=sq[:, 0:nb * 2 * H].rearrange(
                        "p (s e) -> p s e", s=2 * nb),
                    axis=AXX, op=add)
            # rstd = exp(-0.5 * ln(sumsq/H + eps))
            eps_s = cpool.tile([PG, 1], f32)
            nc.vector.memset(eps_s[:], EPS_LN)
            lv = cpool.tile([PG, 2 * NGT], f32)
            nc.scalar.activation(out=lv[:], in_=varT[:], func=LN_F,
                                 bias=eps_s[:], scale=1.0 / H)
            rstd = cpool.tile([PG, 2 * NGT], f32)
            nc.scalar.activation(out=rstd[:], in_=lv[:], func=EXP, scale=-0.5)
            # lm = ln(max(nm, 1))
            lmx = cpool.tile([PG, NGT], f32)
            nc.vector.tensor_scalar_max(lmx[:], nm_s[:], 1.0)
            lm = cpool.tile([PG, NGT], f32)
            nc.scalar.activation(out=lm[:], in_=lmx[:], func=LN_F)
            kl3 = kl_s[:].rearrange("p (g e) -> p g e", g=NGT)
            qq3 = qq_s[:].rearrange("p (g e) -> p g e", g=NGT)
            nc.vector.tensor_copy(kl3[:, :, H:H + 1], lm[:].unsqueeze(2))
            nc.vector.memset(qq3[:, :, H:H + 1], 1.0)
            for g in range(NGT):
                xc = xc_s[:, g * 2 * H:(g + 1) * 2 * H]
                nc.vector.tensor_scalar_mul(
                    kl_s[:, g * EW:g * EW + H], xc[:, 0:H],
                    rstd[:, 2 * g:2 * g + 1])
                nc.vector.tensor_scalar_mul(
                    qq_s[:, g * EW:g * EW + H], xc[:, H:2 * H],
                    rstd[:, 2 * g + 1:2 * g + 2])
            nc.sync.dma_start(out=t_kl, in_=kl_s[:])
            nc.sync.dma_start(out=t_qq, in_=qq_s[:])
    nc.compile()
    _build_cache["A"] = nc
    return nc


# ------------------------------------------------------------------- kernel B
def _build_b(TC, sgs, kpieces, spieces, qpieces):
    key = ("B", TC, tuple(sgs), str(kpieces), str(spieces), str(qpieces))
    if key in _build_cache:
        return _build_cache[key]
    import concourse.bacc as bacc
    import concourse.tile as tile
    import concourse.mybir as mybir
    from concourse.masks import make_identity
    from concourse import library_config

    nc = bacc.Bacc("TRN2", target_bir_lowering=False, debug=False,
                   num_devices=NC)
    f32 = mybir.dt.float32
    f16 = mybir.dt.float16
    bf16 = mybir.dt.bfloat16
    i16 = mybir.dt.int16
    t_ktab = nc.dram_tensor("ktab", [NC * SLAB, EW], f32,
                            kind="ExternalInput").ap()
    t_qtab = nc.dram_tensor("qtab", [SLAB, EW], f32, kind="ExternalInput").ap()
    t_ef2 = nc.dram_tensor("ef2", [PG, TC * H], f16,
                           kind="ExternalInput").ap()
    t_idx3 = nc.dram_tensor("idx3", [PG, TC * 24], i16,
                            kind="ExternalInput").ap()
    t_wu1 = nc.dram_tensor("wu1", [H, H], f16, kind="ExternalInput").ap()
    t_wu2 = nc.dram_tensor("wu2", [H, H], f16, kind="ExternalInput").ap()
    t_hpT = nc.dram_tensor("hpT", [H, RPAD], f16, kind="ExternalInput").ap()
    t_hp2 = nc.dram_tensor("hp2", [PG, NGT * H], f16,
                           kind="ExternalInput").ap()
    t_out = nc.dram_tensor("out", [RPAD, H], f16, kind="ExternalOutput").ap()
    t_acc = nc.dram_tensor("acc", [RPAD, 2 * EW], bf16, kind="Internal").ap()

    mult = mybir.AluOpType.mult
    add = mybir.AluOpType.add
    sub = mybir.AluOpType.subtract
    amax = mybir.AluOpType.max
    AXX = mybir.AxisListType.X
    EXP = mybir.ActivationFunctionType.Exp
    LN_F = mybir.ActivationFunctionType.Ln
    COPY = mybir.ActivationFunctionType.Copy

    with tile.TileContext(nc) as tc, nc.allow_low_precision(reason="fp16"):
        with tc.tile_pool(name="const", bufs=1) as cpool, \
             tc.tile_pool(name="idx", bufs=5) as ipool, \
             tc.tile_pool(name="gat", bufs=5) as gpool, \
             tc.tile_pool(name="wrk", bufs=5) as wpool, \
             tc.tile_pool(name="tl", bufs=6) as tpool, \
             tc.tile_pool(name="ps", bufs=4, space="PSUM") as ppool:
            nc.gpsimd.load_library(library_config.mlp)
            wu1_s = cpool.tile([H, H], f16)
            nc.sync.dma_start(out=wu1_s[:], in_=t_wu1)
            wu2_s = cpool.tile([H, H], f16)
            nc.sync.dma_start(out=wu2_s[:], in_=t_wu2)
            ident = cpool.tile([PG, PG], f16)
            make_identity(nc, ident)
            NZ = NGT * 2 * EW // 7
            z_s = cpool.tile([PG, NZ], bf16)
            nc.vector.memset(z_s[:], 0.0)
            accv = t_acc.rearrange("(p q x) e -> p q (x e)", p=PG, q=7)
            for qq in range(7):
                nc.scalar.dma_start(out=accv[:, qq, :], in_=z_s[:])

            for si, (c0, ncs) in enumerate(sgs):
                idx_t = ipool.tile([PG, ncs * 24], i16, tag="idx")
                nc.sync.dma_start(out=idx_t[:],
                                  in_=t_idx3[:, c0 * 24:(c0 + ncs) * 24])
                kix_t = idx_t[:, 0:ncs * 8]
                qix_t = idx_t[:, ncs * 8:ncs * 16]
                six_t = idx_t[:, ncs * 16:ncs * 24]
                ef_t = wpool.tile([PG, ncs * H], f16, tag="ef")
                nc.sync.dma_start(out=ef_t[:],
                                  in_=t_ef2[:, c0 * H:(c0 + ncs) * H])
                g_k = gpool.tile([PG, ncs * EW], f32, tag="gk")
                for (cp, r0, r1) in kpieces[si]:
                    nc.gpsimd.dma_gather(
                        out_ap=g_k[:, r0 * EW:r1 * EW].rearrange(
                            "p (x e) -> p x e", e=EW),
                        in_ap=t_ktab[cp * SLAB:(cp + 1) * SLAB, :],
                        idxs_ap=kix_t[:, r0 * 8:r1 * 8],
                        num_idxs=(r1 - r0) * PG,
                        num_idxs_reg=(r1 - r0) * PG,
                        elem_size=EW)
                g_q = gpool.tile([PG, ncs * EW], f32, tag="gq")
                for (r0, r1) in qpieces[si]:
                    nc.gpsimd.dma_gather(
                        out_ap=g_q[:, r0 * EW:r1 * EW].rearrange(
                            "p (x e) -> p x e", e=EW),
                        in_ap=t_qtab,
                        idxs_ap=qix_t[:, r0 * 8:r1 * 8],
                        num_idxs=(r1 - r0) * PG,
                        num_idxs_reg=(r1 - r0) * PG,
                        elem_size=EW)
                gk6 = g_k[:].bitcast(f16).rearrange("p (x e) -> p x e", e=2 * EW)
                gq6 = g_q[:].bitcast(f16).rearrange("p (x e) -> p x e", e=2 * EW)
                ef3 = ef_t[:].rearrange("p (x e) -> p x e", e=H)
                kef = wpool.tile([PG, ncs * H], f16, tag="kef")
                kef3 = kef[:].rearrange("p (x e) -> p x e", e=H)
                nc.vector.tensor_tensor(out=kef3, in0=gk6[:, :, 0:H],
                                        in1=ef3, op=add)
                prod = wpool.tile([PG, ncs * H], f16, tag="prod")
                prod3 = prod[:].rearrange("p (x e) -> p x e", e=H)
                nc.vector.tensor_tensor(out=prod3, in0=kef3,
                                        in1=gq6[:, :, 0:H], op=mult)
                score = wpool.tile([PG, ncs], f32, tag="score")
                nc.vector.tensor_reduce(out=score[:].unsqueeze(2), in_=prod3,
                                        axis=AXX, op=add)
                nc.vector.tensor_tensor(out=score[:].unsqueeze(2),
                                        in0=score[:].unsqueeze(2),
                                        in1=gk6[:, :, H:H + 1], op=add)
                esc48 = wpool.tile([PG, ncs * H], bf16, tag="esc48")
                nc.scalar.activation(
                    out=esc48[:].rearrange("p (x e) -> p x e", e=H),
                    in_=score[:].unsqueeze(2).to_broadcast([PG, ncs, H]),
                    func=EXP)
                e48v = esc48[:].rearrange("p (x e) -> p x e", e=H)
                w_t = wpool.tile([PG, ncs * WW], bf16, tag="w")
                w3 = w_t[:].rearrange("p (x e) -> p x e", e=WW)
                nc.vector.tensor_tensor(
                    out=w3[:, :, 0:H], in0=gk6[:, :, EW:EW + H],
                    in1=e48v, op=mult)
                nc.vector.tensor_copy(w3[:, :, H:WW], e48v[:, :, 0:1])
                for (r0, r1) in spieces[si]:
                    nc.gpsimd.dma_scatter_add(
                        out_ap=t_acc[:, 0:WW],
                        in_ap=w3[:, r0:r1, :],
                        idxs_ap=six_t[:, r0 * 8:r1 * 8],
                        num_idxs=(r1 - r0) * PG,
                        num_idxs_reg=(r1 - r0) * PG,
                        elem_size=WW,
                        elem_step=2 * EW)

            # ------------------------------------------------------- tail
            nc.gpsimd.load_library(library_config.standard)
            acc_t = cpool.tile([PG, NGT * 2 * EW], bf16)
            nc.sync.dma_start(
                out=acc_t[:].rearrange("p (x e) -> p x e", e=2 * EW),
                in_=t_acc.rearrange("(p x) e -> p x e", p=PG))
            hpT_s = cpool.tile([H, RPAD], f16)
            nc.sync.dma_start(out=hpT_s[:], in_=t_hpT)
            hp2_s = cpool.tile([PG, NGT * H], f16)
            nc.sync.dma_start(out=hp2_s[:], in_=t_hp2)
            acc3 = acc_t[:].rearrange("p (x e) -> p x e", e=2 * EW)
            den = cpool.tile([PG, NGT], f32)
            nc.vector.tensor_scalar_add(den[:].unsqueeze(2),
                                        acc3[:, :, H:H + 1], EPS_DEN)
            rin = cpool.tile([PG, NGT], f32)
            nc.vector.reciprocal(out=rin[:], in_=den[:])
            r16 = cpool.tile([PG, NGT * H], f16)
            sumT = cpool.tile([PG, NGT], f32)
            varT = cpool.tile([PG, NGT], f32)
            for jj in range(0, NGT, 2):
                agg16 = tpool.tile([PG, 2 * H], f16, tag="agg16")
                for u in (0, 1):
                    j = jj + u
                    nc.vector.tensor_scalar_mul(
                        agg16[:, u * H:(u + 1) * H],
                        acc_t[:, j * 2 * EW:j * 2 * EW + H], rin[:, j:j + 1])
                aggT = ppool.tile([2 * H, PG], f16, tag="aggT")
                nc.tensor.transpose(out=aggT[:], in_=agg16[:],
                                    identity=ident[:])
                aggTs = tpool.tile([2 * H, PG], f16, tag="aggTs")
                nc.scalar.activation(out=aggTs[:], in_=aggT[:], func=COPY)
                zp = ppool.tile([PG, 2 * H], f32, tag="zp")
                for u in (0, 1):
                    j = jj + u
                    zpu = zp[:, u * H:(u + 1) * H]
                    nc.tensor.matmul(out=zpu,
                                     lhsT=hpT_s[:, j * PG:(j + 1) * PG],
                                     rhs=wu1_s[:], start=True, stop=False)
                    nc.tensor.matmul(out=zpu,
                                     lhsT=aggTs[u * H:(u + 1) * H, :],
                                     rhs=wu2_s[:], start=False, stop=True)
                zs = tpool.tile([PG, 2 * H], f16, tag="zs")
                nc.scalar.activation(out=zs[:], in_=zp[:], func=COPY,
                                     scale=0.01)
                z16 = tpool.tile([PG, 2 * H], f16, tag="z16")
                nc.vector.tensor_tensor(out=z16[:], in0=zp[:], in1=zs[:],
                                        op=amax)
                rj2 = r16[:, jj * H:(jj + 2) * H]
                nc.gpsimd.tensor_tensor(out=rj2, in0=z16[:],
                                        in1=hp2_s[:, jj * H:(jj + 2) * H],
                                        op=add)
                sq = tpool.tile([PG, 2 * H], f16, tag="sqt")
                nc.gpsimd.tensor_tensor(out=sq[:], in0=rj2, in1=rj2, op=mult)
                dmy = tpool.tile([PG, H], f16, tag="dmy")
                for u in (0, 1):
                    j = jj + u
                    nc.vector.tensor_scalar(
                        out=dmy[:], in0=r16[:, j * H:(j + 1) * H],
                        scalar1=1.0, scalar2=0.0,
                        op0=mult, op1=add, accum_out=sumT[:, j:j + 1])
                    nc.vector.tensor_scalar(
                        out=dmy[:], in0=sq[:, u * H:(u + 1) * H],
                        scalar1=1.0, scalar2=0.0,
                        op0=mult, op1=add, accum_out=varT[:, j:j + 1])
            mean = cpool.tile([PG, NGT], f32)
            nc.vector.tensor_scalar_mul(mean[:], sumT[:], 1.0 / H)
            m2 = cpool.tile([PG, NGT], f32)
            nc.vector.tensor_tensor(out=m2[:], in0=mean[:], in1=mean[:],
                                    op=mult)
            var = cpool.tile([PG, NGT], f32)
            nc.vector.tensor_scalar_mul(var[:], varT[:], 1.0 / H)
            nc.vector.tensor_tensor(out=var[:], in0=var[:], in1=m2[:], op=sub)
            eps_s = cpool.tile([PG, 1], f32)
            nc.vector.memset(eps_s[:], EPS_LN)
            lv = cpool.tile([PG, NGT], f32)
            nc.scalar.activation(out=lv[:], in_=var[:], func=LN_F,
                                 bias=eps_s[:], scale=1.0)
            rstd = cpool.tile([PG, NGT], f32)
            nc.scalar.activation(out=rstd[:], in_=lv[:], func=EXP, scale=-0.5)
            nmr = cpool.tile([PG, NGT], f32)
            nc.vector.tensor_tensor(out=nmr[:], in0=mean[:], in1=rstd[:],
                                    op=mult)
            nc.vector.tensor_scalar_mul(nmr[:], nmr[:], -1.0)
            on = cpool.tile([PG, NGT * H], f16)
            for j in range(NGT):
                nc.vector.tensor_scalar(
                    out=on[:, j * H:(j + 1) * H], in0=r16[:, j * H:(j + 1) * H],
                    scalar1=rstd[:, j:j + 1], scalar2=nmr[:, j:j + 1],
                    op0=mult, op1=add)
            nc.sync.dma_start(
                out=t_out.rearrange("(p x) e -> p x e", p=PG),
                in_=on[:].rearrange("p (x e) -> p x e", e=H))
    nc.compile()
    _build_cache[key] = nc
    return nc


# -------------------------------------------------------------------- driver
def _make_maps(inputs, prep):
    h = prep["h"]
    wq = np.asarray(inputs["Wq"], np.float64)
    wk = np.asarray(inputs["Wk"], np.float64)
    wm = np.asarray(inputs["Wm"], np.float64)
    wu = np.asarray(inputs["Wu"], np.float64)
    gq = np.asarray(inputs["gq"], np.float32)
    bq = np.asarray(inputs["bq"], np.float32)
    gk = np.asarray(inputs["gk"], np.float32)
    bk = np.asarray(inputs["bk"], np.float32)
    go = np.asarray(inputs["go"], np.float32)
    bo = np.asarray(inputs["bo"], np.float32)
    triv = (np.all(gq == 1) and np.all(gk == 1) and np.all(go == 1)
            and np.all(bq == 0) and np.all(bk == 0) and np.all(bo == 0))
    assert triv, "non-trivial layernorm affine not implemented"

    cen = np.eye(H) - np.full((H, H), 1.0 / H)
    w2 = np.concatenate([wk.T @ cen, wq.T @ cen], axis=1).astype(np.float16)
    wu1 = np.ascontiguousarray(wu[:, :H].T).astype(np.float16)
    wu2 = np.ascontiguousarray((wu[:, H:] @ wm).T).astype(np.float16)

    maps_a = []
    for c in range(NC):
        maps_a.append(dict(hT=prep["hT"][c], w2=w2, nm=prep["nmt"][c]))
    return maps_a, wu1, wu2


def _make_tables(prep, res_a):
    """k-table [NC*SLAB, EW] f32-view (fp16 content) + per-core q tables."""
    h = prep["h"]

    def untile(t):  # [PG, NGT*EW] tile -> [RPAD, EW] rows (node g*128+p)
        return t.reshape(PG, NGT, EW).transpose(1, 0, 2).reshape(RPAD, EW)

    ktab16 = np.zeros((NC, SLAB, 2 * EW), np.float16)
    for c in range(NC):
        kl = untile(res_a[c]["kl"])              # [RPAD, EW] f16
        ktab16[c, :, 0:EW] = kl
        ktab16[c, :BLK, EW:EW + H] = h[c * BLK:(c + 1) * BLK].astype(np.float16)
        ktab16[c, BLK:, H] = LMNEG               # poison rows
    ktab = ktab16.reshape(NC * SLAB, 2 * EW).view(np.float32)
    qtabs = []
    for c in range(NC):
        qq = untile(res_a[c]["qq"])              # [RPAD, EW] f16
        q16 = np.zeros((SLAB, 2 * EW), np.float16)
        q16[:, 0:EW] = qq
        qtabs.append(q16.view(np.float32))
    return ktab, qtabs


def kernel(**inputs):
    from concourse.bass_utils import run_bass_kernel_spmd

    prep = _prep(inputs)
    maps_a, wu1, wu2 = _make_maps(inputs, prep)

    nc_a = _build_a()
    res_a = run_bass_kernel_spmd(nc_a, maps_a, core_ids=list(range(NC))).results

    ktab, qtabs = _make_tables(prep, res_a)

    nc_b = _build_b(prep["TC"], prep["sgs"], prep["kpieces"],
                    prep["spieces"], prep["qpieces"])
    maps_b = []
    for c in range(NC):
        m = dict(prep["in_b"][c])
        m["ktab"] = ktab
        m["qtab"] = qtabs[c]
        m["wu1"] = wu1
        m["wu2"] = wu2
        m["hpT"] = prep["hpT"][c]
        m["hp2"] = prep["hp2"][c]
        maps_b.append(m)
    res_b = run_bass_kernel_spmd(nc_b, maps_b, core_ids=list(range(NC))).results

    out = np.empty((N, H), np.float32)
    for c in range(NC):
        ob = res_b[c]["out"].astype(np.float32)  # [RPAD, H], row = node p*NGT+j
        out[c * BLK:(c + 1) * BLK] = ob[:BLK]
    return out
